# revision 7
# baseline (speedup 1.0000x reference)
"""Trainium2 Bass kernel for BasicEuclideanDistModel log-likelihood (v2).

result = beta*E - sum_e ||dz_uv + dv_uv*t_e + eps|| - dt*sum_{p,j} exp(beta - d_p(t_j))

Architecture (8 cores, SPMD):
- Events dealt to cores round-robin per 128-node u-block so every core has an
  identical per-block count (cap_b); fixed segment structure -> one graph.
- u-side of events/pairs: TensorEngine one-hot gather. Stationary = bf16 table
  block [128 nodes, 4 feats]; moving = host-built fp8 one-hot [128, N].
  PSUM [4, 512] tiles stacked at partition bases {0,32,64}, PE-transposed to
  [128, C] feature planes.
- v-side: SWDGE dma_gather of 256B 4-node blocks (f32), 8192-idx batches on 4
  queues, slot extracted with 4 host mask planes.
- Event d: ||(zu-zv) + (vu-vv)t + eps||; pairs: d^2 = A + t(B + tC) quadratic
  form, exp(beta-d) accumulated on ACT engine.
- Per-core partials [128, 2]; host sums + subtracts pad-dummy contributions.
"""
import os as _os
import sys as _sys
import numpy as np
import ml_dtypes

N_POINTS = 100000
N_RIEMANN = 128
EPS = 1e-6
NON_EVENT_W = 1.0
N_CORES = 8
P = 128
NBLK128 = 782          # 128-node blocks for PE gather
NBLK4 = 25001          # 4-node blocks (+1 zero pad block) for SWDGE
NEV_PAD = 131072       # events per core, padded (16 chunks x 8192)
NCH = 16
CHUNK = 8192
NPR_PAD = 13312        # pairs per core, padded (104 cols)

_cache = {}


def _segments(caps, total):
    """Fixed segment list from per-block capacities.

    Returns list of (c0, c1, blk) covering [0, total), split at block
    boundaries and 512-column PT-tile boundaries. Tail beyond sum(caps)
    is covered with blk=0 (zero one-hots there).
    """
    bounds = []
    off = 0
    for b, cap in enumerate(caps):
        if cap:
            bounds.append((off, off + cap, b))
            off += cap
    assert off <= total
    if off < total:
        bounds.append((off, total, 0))
    segs = []
    for (c0, c1, b) in bounds:
        c = c0
        while c < c1:
            nxt = min(c1, (c // 512 + 1) * 512)
            segs.append((c, nxt, b))
            c = nxt
    return segs


def _build(ev_segs, pr_segs):
    import concourse.bacc as bacc
    import concourse.mybir as mybir
    import concourse.tile as tile
    from concourse import masks as cmasks

    f32 = mybir.dt.float32
    bf16 = mybir.dt.bfloat16
    f8 = mybir.dt.float8e4
    i16 = mybir.dt.int16
    AX = mybir.AxisListType
    OP = mybir.AluOpType
    ACT = mybir.ActivationFunctionType

    nc = bacc.Bacc(num_swdge_queues=4, dynamic_dma_scratch_size=16384)
    tblb_e = nc.declare_dram_parameter("tblb", [P, NBLK128 * 4], bf16, isOutput=False)
    ohu_e = nc.declare_dram_parameter("ohu", [P, NEV_PAD], f8, isOutput=False)
    vbl_e = nc.declare_dram_parameter("vbl", [P, NEV_PAD // 16], i16, isOutput=False)
    te_e = nc.declare_dram_parameter("te", [P, NEV_PAD // P], f32, isOutput=False)
    msk_e = nc.declare_dram_parameter("msk", [P, 4, NEV_PAD // P], f32, isOutput=False)
    tb32_e = nc.declare_dram_parameter("table32", [NBLK4, 64], f32, isOutput=False)
    ohp_e = nc.declare_dram_parameter("ohp", [P, NPR_PAD], f8, isOutput=False)
    pvb_e = nc.declare_dram_parameter("pvb", [P, NPR_PAD // 16], i16, isOutput=False)
    pmsk_e = nc.declare_dram_parameter("pmsk", [P, 4, NPR_PAD // P], f32, isOutput=False)
    t2d_e = nc.declare_dram_parameter("t2d", [P, N_RIEMANN], f32, isOutput=False)
    bt_e = nc.declare_dram_parameter("betac", [1, 1], f32, isOutput=False)
    out_e = nc.declare_dram_parameter("out", [P, 2], f32, isOutput=True)

    PRC = NPR_PAD // P      # 100 pair cols
    ECC = CHUNK // P        # 128 event cols per chunk

    # segment lists grouped by PT tile index
    def segs_by_tile(segs):
        by = {}
        for s in segs:
            by.setdefault(s[0] // 512, []).append(s)
        return by

    ev_by = segs_by_tile(ev_segs)
    pr_by = segs_by_tile(pr_segs)

    with tile.TileContext(nc) as tc:
        with tc.tile_pool(name="persist", bufs=1) as pp, \
             tc.tile_pool(name="oh", bufs=2) as ohp, \
             tc.tile_pool(name="gv", bufs=2) as gvp, \
             tc.tile_pool(name="wk", bufs=2) as wk, \
             tc.tile_pool(name="wp", bufs=2) as wp, \
             tc.tile_pool(name="bk", bufs=2, space="PSUM") as bkp, \
             tc.tile_pool(name="tp", bufs=2, space="PSUM") as tpp:
            tblb = pp.tile([P, NBLK128 * 4], bf16)
            nc.sync.dma_start(out=tblb[:], in_=tblb_e[:])
            te = pp.tile([P, NEV_PAD // P], f32)
            nc.sync.dma_start(out=te[:], in_=te_e[:])
            msk = pp.tile([P, 4, NEV_PAD // P], f32)
            nc.sync.dma_start(out=msk[:], in_=msk_e[:])
            vbl = pp.tile([P, NEV_PAD // 16], i16)
            nc.sync.dma_start(out=vbl[:], in_=vbl_e[:])
            ohpr = pp.tile([P, NPR_PAD], f8)
            nc.sync.dma_start(out=ohpr[:], in_=ohp_e[:])
            pvb = pp.tile([P, NPR_PAD // 16], i16)
            nc.sync.dma_start(out=pvb[:], in_=pvb_e[:])
            pmsk = pp.tile([P, 4, NPR_PAD // P], f32)
            nc.sync.dma_start(out=pmsk[:], in_=pmsk_e[:])
            t2d = pp.tile([P, N_RIEMANN], f32)
            nc.sync.dma_start(out=t2d[:], in_=t2d_e[:])
            bt1 = pp.tile([1, 1], f32)
            nc.sync.dma_start(out=bt1[:], in_=bt_e[:])
            bcol = pp.tile([P, 1], f32)
            nc.gpsimd.partition_broadcast(bcol[:], bt1[:])
            epsc = pp.tile([P, 1], f32)
            nc.vector.memset(epsc[:], EPS)
            ident = pp.tile([P, P], f32)
            cmasks.make_identity(nc, ident[:])

            acc_ev = pp.tile([P, NCH], f32)
            acc_ne = pp.tile([P, PRC], f32)

            # ---------- PE one-hot gather -> feature planes ----------
            def pe_gather(oh_tile, by_tile, tiles0, ntiles, planes):
                """Gather ntiles PT tiles (512 ev each) starting at global PT
                index tiles0, one-hot from oh_tile (chunk-local cols), into
                planes[f] [128, 4*ntiles] (chunk-local)."""
                g = 0
                while g * 3 < ntiles:
                    tl = [3 * g + i for i in range(3) if 3 * g + i < ntiles]
                    bank = bkp.tile([P, 512], f32, tag="bank")
                    for bi, t in enumerate(tl):
                        for (c0, c1, b) in by_tile.get(tiles0 + t, []):
                            lc0 = c0 - (tiles0 + t) * 512
                            oc0 = c0 - tiles0 * 512
                            nc.tensor.matmul(
                                out=bank[32 * bi:32 * bi + 4, lc0:lc0 + (c1 - c0)],
                                lhsT=tblb[:, 4 * b:4 * b + 4],
                                rhs=oh_tile[:, oc0:oc0 + (c1 - c0)],
                                start=True, stop=True)
                    stack = wk.tile([P, 512], f32, tag="stack")
                    nc.vector.tensor_copy(out=stack[:], in_=bank[:])
                    tb = tpp.tile([P, 4, 4, 32], f32, tag="tb")
                    for j in range(4):
                        nc.tensor.transpose(out=tb[:, j], in_=stack[:, P * j:P * j + P],
                                            identity=ident[:])
                    for bi, t in enumerate(tl):
                        for f in range(4):
                            nc.vector.tensor_copy(
                                out=planes[f][:, 4 * t:4 * t + 4],
                                in_=tb[:, :, bi, f])
                    g += 1

            # ---------- events ----------
            for c in range(NCH):
                ohu_t = ohp.tile([P, CHUNK], f8, tag="ohu")
                nc.sync.dma_start(out=ohu_t[:], in_=ohu_e[:, c * CHUNK:(c + 1) * CHUNK])
                gv = gvp.tile([P, ECC, 64], f32, tag="gv")
                q0 = c * CHUNK
                nc.gpsimd.dma_gather(
                    out_ap=gv[:], in_ap=tb32_e[:],
                    idxs_ap=vbl[:, q0 // 16:(q0 + CHUNK) // 16],
                    num_idxs=CHUNK, num_idxs_reg=CHUNK, elem_size=64,
                    single_packet=False, queue_num=c % 4)
                planes = [wk.tile([P, ECC], f32, tag=f"fu{f}", name=f"fu{f}") for f in range(4)]
                pe_gather(ohu_t, ev_by, c * (CHUNK // 512), CHUNK // 512, planes)
                # v-side slot select + d compute
                mc = msk[:, :, c * ECC:(c + 1) * ECC]
                tec = te[:, c * ECC:(c + 1) * ECC]
                dd = []
                for f in range(4):
                    sel = wp.tile([P, ECC], f32, tag="sel")
                    tmp = wp.tile([P, ECC], f32, tag="seltmp")
                    nc.vector.tensor_tensor(out=sel[:], in0=mc[:, 0],
                                            in1=gv[:, :, f], op=OP.mult)
                    for s in range(1, 4):
                        nc.vector.tensor_tensor(out=tmp[:], in0=mc[:, s],
                                                in1=gv[:, :, 16 * s + f], op=OP.mult)
                        nc.vector.tensor_tensor(out=sel[:], in0=sel[:], in1=tmp[:],
                                                op=OP.add)
                    d = wp.tile([P, ECC], f32, tag=f"d{f}")
                    nc.vector.tensor_tensor(out=d[:], in0=planes[f][:], in1=sel[:],
                                            op=OP.subtract)
                    dd.append(d)
                dzx, dzy, dvx, dvy = dd
                wx = wp.tile([P, ECC], f32, tag="wx")
                nc.vector.tensor_tensor(out=wx[:], in0=dvx[:], in1=tec, op=OP.mult)
                nc.vector.tensor_tensor(out=wx[:], in0=wx[:], in1=dzx[:], op=OP.add)
                wy = wp.tile([P, ECC], f32, tag="wy")
                nc.vector.tensor_tensor(out=wy[:], in0=dvy[:], in1=tec, op=OP.mult)
                nc.vector.tensor_tensor(out=wy[:], in0=wy[:], in1=dzy[:], op=OP.add)
                sx = wp.tile([P, ECC], f32, tag="sx")
                nc.scalar.activation(sx[:], wx[:], ACT.Square, bias=epsc[:])
                sy = wp.tile([P, ECC], f32, tag="sy")
                nc.scalar.activation(sy[:], wy[:], ACT.Square, bias=epsc[:])
                nc.vector.tensor_tensor(out=sx[:], in0=sx[:], in1=sy[:], op=OP.add)
                dt_ = wp.tile([P, ECC], f32, tag="dt")
                nc.scalar.activation(dt_[:], sx[:], ACT.Sqrt,
                                     accum_out=acc_ev[:, c:c + 1])

            # ---------- pairs ----------
            pplanes = [pp.tile([P, PRC], f32, name=f"fp{f}") for f in range(4)]
            pe_gather(ohpr, pr_by, 0, NPR_PAD // 512, pplanes)
            pd = [pp.tile([P, PRC], f32, name=f"pd{f}") for f in range(4)]
            for h in range(2):
                gpv = gvp.tile([P, 64, 64], f32, tag="gv", name="gpv")
                q0 = h * 6656
                nc.gpsimd.dma_gather(
                    out_ap=gpv[:, 0:52, :], in_ap=tb32_e[:],
                    idxs_ap=pvb[:, q0 // 16:(q0 + 6656) // 16],
                    num_idxs=6656, num_idxs_reg=6656, elem_size=64,
                    single_packet=False, queue_num=h)
                hs = slice(52 * h, 52 * (h + 1))
                for f in range(4):
                    sel = wp.tile([P, 52], f32, tag="psel")
                    tmp = wp.tile([P, 52], f32, tag="pseltmp")
                    nc.vector.tensor_tensor(out=sel[:], in0=pmsk[:, 0, hs],
                                            in1=gpv[:, 0:52, f], op=OP.mult)
                    for s in range(1, 4):
                        nc.vector.tensor_tensor(out=tmp[:], in0=pmsk[:, s, hs],
                                                in1=gpv[:, 0:52, 16 * s + f], op=OP.mult)
                        nc.vector.tensor_tensor(out=sel[:], in0=sel[:], in1=tmp[:],
                                                op=OP.add)
                    nc.vector.tensor_tensor(out=pd[f][:, hs], in0=pplanes[f][:, hs],
                                            in1=sel[:], op=OP.subtract)
            dzx, dzy, dvx, dvy = pd
            # A = (dz+eps)^2 sum; B = 2 (dz+eps).dv; C = dv.dv
            px = pp.tile([P, PRC], f32)
            nc.vector.tensor_tensor(out=px[:], in0=dzx[:],
                                    in1=epsc[:].to_broadcast(dzx.shape), op=OP.add)
            py = pp.tile([P, PRC], f32)
            nc.vector.tensor_tensor(out=py[:], in0=dzy[:],
                                    in1=epsc[:].to_broadcast(dzy.shape), op=OP.add)
            At = pp.tile([P, PRC], f32)
            Bt = pp.tile([P, PRC], f32)
            Ct = pp.tile([P, PRC], f32)
            t1 = wp.tile([P, PRC], f32, tag="pt1")
            nc.vector.tensor_tensor(out=At[:], in0=px[:], in1=px[:], op=OP.mult)
            nc.vector.tensor_tensor(out=t1[:], in0=py[:], in1=py[:], op=OP.mult)
            nc.vector.tensor_tensor(out=At[:], in0=At[:], in1=t1[:], op=OP.add)
            nc.vector.tensor_tensor(out=Bt[:], in0=px[:], in1=dvx[:], op=OP.mult)
            nc.vector.tensor_tensor(out=t1[:], in0=py[:], in1=dvy[:], op=OP.mult)
            nc.vector.tensor_tensor(out=Bt[:], in0=Bt[:], in1=t1[:], op=OP.add)
            nc.vector.tensor_tensor(out=Bt[:], in0=Bt[:], in1=Bt[:], op=OP.add)
            nc.vector.tensor_tensor(out=Ct[:], in0=dvx[:], in1=dvx[:], op=OP.mult)
            nc.vector.tensor_tensor(out=t1[:], in0=dvy[:], in1=dvy[:], op=OP.mult)
            nc.vector.tensor_tensor(out=Ct[:], in0=Ct[:], in1=t1[:], op=OP.add)
            for k in range(PRC):
                w1 = wp.tile([P, N_RIEMANN], f32, tag="w1")
                nc.vector.tensor_tensor(
                    out=w1[:], in0=t2d[:],
                    in1=Ct[:, k:k + 1].to_broadcast([P, N_RIEMANN]), op=OP.mult)
                nc.vector.tensor_tensor(
                    out=w1[:], in0=w1[:],
                    in1=Bt[:, k:k + 1].to_broadcast([P, N_RIEMANN]), op=OP.add)
                nc.vector.tensor_tensor(out=w1[:], in0=w1[:], in1=t2d[:], op=OP.mult)
                nc.vector.tensor_tensor(
                    out=w1[:], in0=w1[:],
                    in1=At[:, k:k + 1].to_broadcast([P, N_RIEMANN]), op=OP.add)
                w2 = wp.tile([P, N_RIEMANN], f32, tag="w2")
                nc.scalar.activation(w2[:], w1[:], ACT.Sqrt)
                w3 = wp.tile([P, N_RIEMANN], f32, tag="w3")
                nc.scalar.activation(w3[:], w2[:], ACT.Exp, bias=bcol[:],
                                     scale=-1.0, accum_out=acc_ne[:, k:k + 1])

            res = pp.tile([P, 2], f32)
            nc.vector.tensor_reduce(res[:, 0:1], acc_ev[:], axis=AX.X, op=OP.add)
            nc.vector.tensor_reduce(res[:, 1:2], acc_ne[:], axis=AX.X, op=OP.add)
            nc.sync.dma_start(out=out_e[:], in_=res[:])

    nc.compile()
    return nc


def _wrap16(blk):
    w = blk.reshape(-1, 16).T
    return np.tile(w, (8, 1)).astype(np.int16)


def _plane(arr, dtype=np.float32):
    return np.ascontiguousarray(arr.reshape(-1, P).T).astype(dtype)


def _deal(blk_ids, nblk, ncores, cap_round=1):
    """Deal items to cores per block, round-robin. Returns (per-core position
    lists, caps). Items of block b on core c land at a common offset table."""
    order = np.argsort(blk_ids, kind="stable")
    counts = np.bincount(blk_ids, minlength=nblk)
    caps = (counts + ncores - 1) // ncores
    return order, counts, caps


def kernel(beta, z0, v0, a0, u, v, event_times, pair_u, pair_v, t0, tn):
    assert not np.any(np.asarray(a0)), "kernel assumes a0 == 0"
    beta = np.asarray(beta, np.float32)
    z0 = np.asarray(z0, np.float32)
    v0 = np.asarray(v0, np.float32)
    u = np.asarray(u).astype(np.int64)
    v = np.asarray(v).astype(np.int64)
    event_times = np.asarray(event_times, np.float32)
    pair_u = np.asarray(pair_u).astype(np.int64)
    pair_v = np.asarray(pair_v).astype(np.int64)
    t0f = float(np.asarray(t0))
    tnf = float(np.asarray(tn))
    b = float(beta.reshape(-1)[0])
    E = u.shape[0]
    NPAIR = pair_u.shape[0]

    # ---- deal events/pairs to cores by u-block ----
    ublk = (u // P).astype(np.int64)
    order, counts, caps = _deal(ublk, NBLK128, N_CORES)
    assert caps.sum() <= NEV_PAD, caps.sum()
    pblk = (pair_u // P).astype(np.int64)
    porder, pcounts, pcaps = _deal(pblk, NBLK128, N_CORES)
    assert pcaps.sum() <= NPR_PAD, pcaps.sum()

    ev_segs = _segments(caps, NEV_PAD)
    pr_segs = _segments(pcaps, NPR_PAD)
    key = (tuple(caps), tuple(pcaps))
    if key not in _cache:
        _cache[key] = _build(ev_segs, pr_segs)
    nc = _cache[key]

    # per-core slot assignment: block b's events at cols [off_b, off_b+cap_b)
    offs = np.zeros(NBLK128 + 1, np.int64)
    offs[1:] = np.cumsum(caps)
    poffs = np.zeros(NBLK128 + 1, np.int64)
    poffs[1:] = np.cumsum(pcaps)

    # positions: sorted events of block b occupy order[cstart_b : cstart_b+counts_b];
    # deal index j -> core j % 8, slot off_b + j // 8
    cstart = np.zeros(NBLK128 + 1, np.int64)
    cstart[1:] = np.cumsum(counts)
    pcstart = np.zeros(NBLK128 + 1, np.int64)
    pcstart[1:] = np.cumsum(pcounts)

    def placements(order_, counts_, cstart_, offs_, npad):
        """-> core id and padded position for each original item (by sorted order)."""
        n = order_.shape[0]
        j_in_blk = np.arange(n) - np.repeat(cstart_[:-1], counts_)
        cores = j_in_blk % N_CORES
        slots = np.repeat(offs_[:-1], counts_) + j_in_blk // N_CORES
        return cores, slots  # aligned with order_

    ev_cores, ev_slots = placements(order, counts, cstart, offs, NEV_PAD)
    pr_cores, pr_slots = placements(porder, pcounts, pcstart, poffs, NPR_PAD)

    # ---- tables ----
    tblb = np.zeros((P, NBLK128 * 4), np.float32)
    n = np.arange(N_POINTS)
    tblb[n % P, 4 * (n // P) + 0] = z0[:, 0]
    tblb[n % P, 4 * (n // P) + 1] = z0[:, 1]
    tblb[n % P, 4 * (n // P) + 2] = v0[:, 0]
    tblb[n % P, 4 * (n // P) + 3] = v0[:, 1]
    tblb = tblb.astype(ml_dtypes.bfloat16)

    tb32 = np.zeros((NBLK4 * 4, 16), np.float32)
    tb32[:N_POINTS, 0:2] = z0
    tb32[:N_POINTS, 2:4] = v0
    tb32 = np.ascontiguousarray(tb32.reshape(NBLK4, 64))

    dt = (tnf - t0f) / N_RIEMANN
    ts = (t0f + (np.arange(N_RIEMANN, dtype=np.float32) / N_RIEMANN)
          * (tnf - t0f)).astype(np.float32)
    t2d = np.tile(ts[None, :], (P, 1))

    in_maps = []
    for c in range(N_CORES):
        sel = ev_cores == c
        so = order[sel]              # original event ids for this core
        slots = ev_slots[sel]
        uu = np.zeros(NEV_PAD, np.int64)
        vv = np.full(NEV_PAD, 4 * (NBLK4 - 1), np.int64)  # pad -> zero block
        tt = np.zeros(NEV_PAD, np.float32)
        live = np.zeros(NEV_PAD, bool)
        uu[slots] = u[so]
        vv[slots] = v[so]
        tt[slots] = event_times[so]
        live[slots] = True
        ohu = np.zeros((P, NEV_PAD), ml_dtypes.float8_e4m3fn)
        ohu[uu[live] % P, np.flatnonzero(live)] = 1.0
        mskf = np.zeros((4, NEV_PAD), np.float32)
        mskf[vv[live] % 4, np.flatnonzero(live)] = 1.0
        msk = np.stack([_plane(mskf[s]) for s in range(4)], axis=1)

        psel = pr_cores == c
        pso = porder[psel]
        pslots = pr_slots[psel]
        pu_ = np.zeros(NPR_PAD, np.int64)
        pv_ = np.full(NPR_PAD, 4 * (NBLK4 - 1), np.int64)
        plive = np.zeros(NPR_PAD, bool)
        pu_[pslots] = pair_u[pso]
        pv_[pslots] = pair_v[pso]
        plive[pslots] = True
        ohpr = np.zeros((P, NPR_PAD), ml_dtypes.float8_e4m3fn)
        ohpr[pu_[plive] % P, np.flatnonzero(plive)] = 1.0
        pmskf = np.zeros((4, NPR_PAD), np.float32)
        pmskf[pv_[plive] % 4, np.flatnonzero(plive)] = 1.0
        pmsk = np.stack([_plane(pmskf[s]) for s in range(4)], axis=1)

        in_maps.append({
            "tblb": tblb,
            "ohu": ohu,
            "vbl": _wrap16(vv // 4),
            "te": _plane(tt),
            "msk": msk,
            "table32": tb32,
            "ohp": ohpr,
            "pvb": _wrap16(pv_ // 4),
            "pmsk": pmsk,
            "t2d": t2d,
            "betac": np.full((1, 1), b, np.float32),
        })

    trace = bool(_os.environ.get("KERNEL_TRACE"))
    if trace:
        try:
            import types
            if "antenv.axon_hooks" not in _sys.modules:
                mod = types.ModuleType("antenv.axon_hooks")
                mod._hook = None
                mod.set_axon_ntff_profile_hook = lambda h: setattr(mod, "_hook", h)
                mod.get_axon_ntff_profile_hook = lambda: mod._hook
                import antenv
                antenv.axon_hooks = mod
                _sys.modules["antenv.axon_hooks"] = mod
                from trn_agent_boot.trn_boot import _ntff_profile_via_ctypes
                hk = _ntff_profile_via_ctypes("/opt/axon/libaxon_pjrt.so")
                if hk is not None:
                    mod.set_axon_ntff_profile_hook(hk)
        except Exception:
            trace = False
    from concourse.bass_utils import run_bass_kernel_spmd
    r = run_bass_kernel_spmd(nc, in_maps, core_ids=list(range(N_CORES)),
                             trace=trace)
    globals()["LAST_EXEC_NS"] = r.exec_time_ns

    ev_sum = 0.0
    ne_sum = 0.0
    for c in range(N_CORES):
        out = r.results[c]["out"].astype(np.float64)
        ev_sum += out[:, 0].sum()
        ne_sum += out[:, 1].sum()

    # pad corrections: event pad -> d = sqrt(2)*eps; pair pad -> per t step
    d_dummy = np.sqrt(2.0) * EPS
    n_ev_dummy = N_CORES * NEV_PAD - E
    n_pr_dummy = N_CORES * NPR_PAD - NPAIR
    ev_sum -= n_ev_dummy * d_dummy
    ne_sum -= n_pr_dummy * N_RIEMANN * np.exp(b - d_dummy)

    globals()["DEBUG_PARTS"] = (ev_sum, ne_sum)
    result = b * E - ev_sum - NON_EVENT_W * ne_sum * dt
    return np.float32(result)


# revision 9
# speedup vs baseline: 1.0131x; 1.0131x over previous
"""Trainium2 Bass kernel for BasicEuclideanDistModel log-likelihood (v2).

result = beta*E - sum_e ||dz_uv + dv_uv*t_e + eps|| - dt*sum_{p,j} exp(beta - d_p(t_j))

Architecture (8 cores, SPMD):
- Events dealt to cores round-robin per 128-node u-block so every core has an
  identical per-block count (cap_b); fixed segment structure -> one graph.
- u-side of events/pairs: TensorEngine one-hot gather. Stationary = bf16 table
  block [128 nodes, 4 feats]; moving = host-built fp8 one-hot [128, N].
  PSUM [4, 512] tiles stacked at partition bases {0,32,64}, PE-transposed to
  [128, C] feature planes.
- v-side: SWDGE dma_gather of 256B 4-node blocks (f32), 8192-idx batches on 4
  queues, slot extracted with 4 host mask planes.
- Event d: ||(zu-zv) + (vu-vv)t + eps||; pairs: d^2 = A + t(B + tC) quadratic
  form, exp(beta-d) accumulated on ACT engine.
- Per-core partials [128, 2]; host sums + subtracts pad-dummy contributions.
"""
import os as _os
import sys as _sys
import numpy as np
import ml_dtypes

N_POINTS = 100000
N_RIEMANN = 128
EPS = 1e-6
NON_EVENT_W = 1.0
N_CORES = 8
P = 128
NBLK128 = 782          # 128-node blocks for PE gather
NBLK4 = 25001          # 4-node blocks (+1 zero pad block) for SWDGE
NEV_PAD = 131072       # events per core, padded (16 chunks x 8192)
NCH = 16
CHUNK = 8192
NPR_PAD = 13312        # pairs per core, padded (104 cols)

_cache = {}


def _segments(caps, total):
    """Fixed segment list from per-block capacities.

    Returns list of (c0, c1, blk) covering [0, total), split at block
    boundaries and 512-column PT-tile boundaries. Tail beyond sum(caps)
    is covered with blk=0 (zero one-hots there).
    """
    bounds = []
    off = 0
    for b, cap in enumerate(caps):
        if cap:
            bounds.append((off, off + cap, b))
            off += cap
    assert off <= total
    if off < total:
        bounds.append((off, total, 0))
    segs = []
    for (c0, c1, b) in bounds:
        c = c0
        while c < c1:
            nxt = min(c1, (c // 512 + 1) * 512)
            segs.append((c, nxt, b))
            c = nxt
    return segs


def _build(ev_segs, pr_segs):
    import concourse.bacc as bacc
    import concourse.mybir as mybir
    import concourse.tile as tile
    from concourse import masks as cmasks

    f32 = mybir.dt.float32
    bf16 = mybir.dt.bfloat16
    f8 = mybir.dt.float8e4
    i16 = mybir.dt.int16
    i8 = mybir.dt.int8
    AX = mybir.AxisListType
    OP = mybir.AluOpType
    ACT = mybir.ActivationFunctionType

    nc = bacc.Bacc(num_swdge_queues=4, dynamic_dma_scratch_size=16384)
    tblb_e = nc.declare_dram_parameter("tblb", [P, NBLK128 * 4], bf16, isOutput=False)
    ohu_e = nc.declare_dram_parameter("ohu", [P, NEV_PAD], f8, isOutput=False)
    vbl_e = nc.declare_dram_parameter("vbl", [P, NEV_PAD // 16], i16, isOutput=False)
    te_e = nc.declare_dram_parameter("te", [P, NEV_PAD // P], f32, isOutput=False)
    msk_e = nc.declare_dram_parameter("msk", [P, 4, NEV_PAD // P], i8, isOutput=False)
    tb32_e = nc.declare_dram_parameter("table32", [NBLK4, 64], f32, isOutput=False)
    ohp_e = nc.declare_dram_parameter("ohp", [P, NPR_PAD], f8, isOutput=False)
    pvb_e = nc.declare_dram_parameter("pvb", [P, NPR_PAD // 16], i16, isOutput=False)
    pmsk_e = nc.declare_dram_parameter("pmsk", [P, 4, NPR_PAD // P], i8, isOutput=False)
    t2d_e = nc.declare_dram_parameter("t2d", [P, N_RIEMANN], f32, isOutput=False)
    t2g_e = nc.declare_dram_parameter("t2g", [P, 13, N_RIEMANN], f32, isOutput=False)
    bt_e = nc.declare_dram_parameter("betac", [1, 1], f32, isOutput=False)
    out_e = nc.declare_dram_parameter("out", [P, 2], f32, isOutput=True)

    PRC = NPR_PAD // P      # 100 pair cols
    ECC = CHUNK // P        # 128 event cols per chunk

    # segment lists grouped by PT tile index
    def segs_by_tile(segs):
        by = {}
        for s in segs:
            by.setdefault(s[0] // 512, []).append(s)
        return by

    ev_by = segs_by_tile(ev_segs)
    pr_by = segs_by_tile(pr_segs)

    with tile.TileContext(nc) as tc:
        with tc.tile_pool(name="persist", bufs=1) as pp, \
             tc.tile_pool(name="oh", bufs=3) as ohp, \
             tc.tile_pool(name="gv", bufs=4) as gvp, \
             tc.tile_pool(name="wk", bufs=2) as wk, \
             tc.tile_pool(name="wp", bufs=2) as wp, \
             tc.tile_pool(name="bk", bufs=2, space="PSUM") as bkp, \
             tc.tile_pool(name="tp", bufs=2, space="PSUM") as tpp:
            tblb = pp.tile([P, NBLK128 * 4], bf16)
            nc.sync.dma_start(out=tblb[:], in_=tblb_e[:])
            te = pp.tile([P, NEV_PAD // P], f32)
            nc.sync.dma_start(out=te[:], in_=te_e[:])
            msk = pp.tile([P, 4, NEV_PAD // P], i8)
            nc.sync.dma_start(out=msk[:], in_=msk_e[:])
            vbl = pp.tile([P, NEV_PAD // 16], i16)
            nc.sync.dma_start(out=vbl[:], in_=vbl_e[:])
            ohpr = pp.tile([P, NPR_PAD], f8)
            nc.sync.dma_start(out=ohpr[:], in_=ohp_e[:])
            pvb = pp.tile([P, NPR_PAD // 16], i16)
            nc.sync.dma_start(out=pvb[:], in_=pvb_e[:])
            pmsk = pp.tile([P, 4, NPR_PAD // P], i8)
            nc.sync.dma_start(out=pmsk[:], in_=pmsk_e[:])
            t2d = pp.tile([P, N_RIEMANN], f32)
            nc.sync.dma_start(out=t2d[:], in_=t2d_e[:])
            t2g = pp.tile([P, 13, N_RIEMANN], f32)
            nc.sync.dma_start(out=t2g[:], in_=t2g_e[:])
            bt1 = pp.tile([1, 1], f32)
            nc.sync.dma_start(out=bt1[:], in_=bt_e[:])
            bcol = pp.tile([P, 1], f32)
            nc.gpsimd.partition_broadcast(bcol[:], bt1[:])
            epsc = pp.tile([P, 1], f32)
            nc.vector.memset(epsc[:], EPS)
            ident = pp.tile([P, P], f32)
            cmasks.make_identity(nc, ident[:])

            acc_ev = pp.tile([P, NCH], f32)
            acc_ne = pp.tile([P, PRC // 13], f32)

            # ---------- PE one-hot gather -> feature planes ----------
            def pe_gather(oh_tile, by_tile, tiles0, ntiles, ftile):
                """Gather ntiles PT tiles (512 ev each) from global PT index
                tiles0 into ftile [128, 4*ntiles cols, 4 feats]."""
                g = 0
                while g * 3 < ntiles:
                    tl = [3 * g + i for i in range(3) if 3 * g + i < ntiles]
                    bank = bkp.tile([P, 512], f32, tag="bank")
                    for bi, t in enumerate(tl):
                        for (c0, c1, b) in by_tile.get(tiles0 + t, []):
                            lc0 = c0 - (tiles0 + t) * 512
                            oc0 = c0 - tiles0 * 512
                            nc.tensor.matmul(
                                out=bank[32 * bi:32 * bi + 4, lc0:lc0 + (c1 - c0)],
                                lhsT=tblb[:, 4 * b:4 * b + 4],
                                rhs=oh_tile[:, oc0:oc0 + (c1 - c0)],
                                start=True, stop=True)
                    stack = wk.tile([P, 512], f32, tag="stack")
                    nc.vector.tensor_copy(out=stack[:], in_=bank[:])
                    tb = tpp.tile([P, 4, 4, 32], f32, tag="tb")
                    for j in range(4):
                        nc.tensor.transpose(out=tb[:, j], in_=stack[:, P * j:P * j + P],
                                            identity=ident[:])
                    for bi, t in enumerate(tl):
                        nc.vector.tensor_copy(
                            out=ftile[:, 4 * t:4 * t + 4, :],
                            in_=tb[:, :, bi, 0:4])
                    g += 1

            # ---------- events ----------
            for c in range(NCH):
                ohu_t = ohp.tile([P, CHUNK], f8, tag="ohu")
                nc.sync.dma_start(out=ohu_t[:], in_=ohu_e[:, c * CHUNK:(c + 1) * CHUNK])
                gv = gvp.tile([P, ECC, 64], f32, tag="gv")
                q0 = c * CHUNK
                nc.gpsimd.dma_gather(
                    out_ap=gv[:], in_ap=tb32_e[:],
                    idxs_ap=vbl[:, q0 // 16:(q0 + CHUNK) // 16],
                    num_idxs=CHUNK, num_idxs_reg=CHUNK, elem_size=64,
                    single_packet=False, queue_num=c % 4)
                F = wk.tile([P, ECC, 4], f32, tag="F")
                pe_gather(ohu_t, ev_by, c * (CHUNK // 512), CHUNK // 512, F)
                # v-side slot select + d compute
                mc = msk[:, :, c * ECC:(c + 1) * ECC]
                tec = te[:, c * ECC:(c + 1) * ECC]
                dd = []
                for f in range(4):
                    sel = wp.tile([P, ECC], f32, tag=f"sel{f}", name=f"sel{f}")
                    nc.vector.tensor_copy(out=sel[:], in_=gv[:, :, f])
                    for s in range(1, 4):
                        nc.vector.copy_predicated(sel[:], mc[:, s],
                                                  gv[:, :, 16 * s + f])
                    d = wp.tile([P, ECC], f32, tag=f"d{f}", name=f"d{f}")
                    nc.vector.tensor_tensor(out=d[:], in0=F[:, :, f], in1=sel[:],
                                            op=OP.subtract)
                    dd.append(d)
                dzx, dzy, dvx, dvy = dd
                ws = wp.tile([P, 2, ECC], f32, tag="ws")
                nc.vector.tensor_tensor(out=ws[:, 0], in0=dvx[:], in1=tec, op=OP.mult)
                nc.vector.tensor_tensor(out=ws[:, 0], in0=ws[:, 0], in1=dzx[:],
                                        op=OP.add)
                nc.vector.tensor_tensor(out=ws[:, 1], in0=dvy[:], in1=tec, op=OP.mult)
                nc.vector.tensor_tensor(out=ws[:, 1], in0=ws[:, 1], in1=dzy[:],
                                        op=OP.add)
                sq = wp.tile([P, 2, ECC], f32, tag="sq")
                nc.scalar.activation(sq[:], ws[:], ACT.Square, bias=epsc[:])
                nc.vector.tensor_tensor(out=sq[:, 0], in0=sq[:, 0], in1=sq[:, 1],
                                        op=OP.add)
                dt_ = wp.tile([P, ECC], f32, tag="dt")
                nc.scalar.activation(dt_[:], sq[:, 0], ACT.Sqrt,
                                     accum_out=acc_ev[:, c:c + 1])

            # ---------- pairs ----------
            pF = pp.tile([P, PRC, 4], f32)
            pe_gather(ohpr, pr_by, 0, NPR_PAD // 512, pF)
            pd = [pp.tile([P, PRC], f32, name=f"pd{f}") for f in range(4)]
            for h in range(2):
                gpv = gvp.tile([P, 64, 64], f32, tag="gv", name="gpv")
                q0 = h * 6656
                nc.gpsimd.dma_gather(
                    out_ap=gpv[:, 0:52, :], in_ap=tb32_e[:],
                    idxs_ap=pvb[:, q0 // 16:(q0 + 6656) // 16],
                    num_idxs=6656, num_idxs_reg=6656, elem_size=64,
                    single_packet=False, queue_num=h)
                hs = slice(52 * h, 52 * (h + 1))
                for f in range(4):
                    sel = wp.tile([P, 52], f32, tag="psel")
                    nc.vector.tensor_copy(out=sel[:], in_=gpv[:, 0:52, f])
                    for s in range(1, 4):
                        nc.vector.copy_predicated(sel[:], pmsk[:, s, hs],
                                                  gpv[:, 0:52, 16 * s + f])
                    nc.vector.tensor_tensor(out=pd[f][:, hs], in0=pF[:, hs, f],
                                            in1=sel[:], op=OP.subtract)
            dzx, dzy, dvx, dvy = pd
            # A = (dz+eps)^2 sum; B = 2 (dz+eps).dv; C = dv.dv
            px = pp.tile([P, PRC], f32)
            nc.vector.tensor_tensor(out=px[:], in0=dzx[:],
                                    in1=epsc[:].to_broadcast(dzx.shape), op=OP.add)
            py = pp.tile([P, PRC], f32)
            nc.vector.tensor_tensor(out=py[:], in0=dzy[:],
                                    in1=epsc[:].to_broadcast(dzy.shape), op=OP.add)
            At = pp.tile([P, PRC, 1], f32)
            Bt = pp.tile([P, PRC, 1], f32)
            Ct = pp.tile([P, PRC, 1], f32)
            t1 = wp.tile([P, PRC], f32, tag="pt1")
            nc.vector.tensor_tensor(out=At[:, :, 0], in0=px[:], in1=px[:], op=OP.mult)
            nc.vector.tensor_tensor(out=t1[:], in0=py[:], in1=py[:], op=OP.mult)
            nc.vector.tensor_tensor(out=At[:, :, 0], in0=At[:, :, 0], in1=t1[:],
                                    op=OP.add)
            nc.vector.tensor_tensor(out=Bt[:, :, 0], in0=px[:], in1=dvx[:], op=OP.mult)
            nc.vector.tensor_tensor(out=t1[:], in0=py[:], in1=dvy[:], op=OP.mult)
            nc.vector.tensor_tensor(out=Bt[:, :, 0], in0=Bt[:, :, 0], in1=t1[:],
                                    op=OP.add)
            nc.vector.tensor_tensor(out=Bt[:, :, 0], in0=Bt[:, :, 0], in1=Bt[:, :, 0],
                                    op=OP.add)
            nc.vector.tensor_tensor(out=Ct[:, :, 0], in0=dvx[:], in1=dvx[:], op=OP.mult)
            nc.vector.tensor_tensor(out=t1[:], in0=dvy[:], in1=dvy[:], op=OP.mult)
            nc.vector.tensor_tensor(out=Ct[:, :, 0], in0=Ct[:, :, 0], in1=t1[:],
                                    op=OP.add)
            GW = 13
            for q in range(PRC // GW):
                ks = slice(q * GW, (q + 1) * GW)
                shp = [P, GW, N_RIEMANN]
                w1 = wp.tile(shp, f32, tag="w1", bufs=1)
                nc.vector.tensor_tensor(out=w1[:], in0=t2g[:],
                                        in1=Ct[:, ks, :].to_broadcast(shp),
                                        op=OP.mult)
                nc.vector.tensor_tensor(out=w1[:], in0=w1[:],
                                        in1=Bt[:, ks, :].to_broadcast(shp),
                                        op=OP.add)
                nc.vector.tensor_tensor(out=w1[:], in0=w1[:], in1=t2g[:], op=OP.mult)
                nc.vector.tensor_tensor(out=w1[:], in0=w1[:],
                                        in1=At[:, ks, :].to_broadcast(shp),
                                        op=OP.add)
                w2 = wp.tile(shp, f32, tag="w2", bufs=1)
                nc.scalar.activation(w2[:], w1[:], ACT.Sqrt)
                w3 = wp.tile(shp, f32, tag="w3", bufs=1)
                nc.scalar.activation(w3[:], w2[:], ACT.Exp, bias=bcol[:],
                                     scale=-1.0, accum_out=acc_ne[:, q:q + 1])

            res = pp.tile([P, 2], f32)
            nc.vector.tensor_reduce(res[:, 0:1], acc_ev[:], axis=AX.X, op=OP.add)
            nc.vector.tensor_reduce(res[:, 1:2], acc_ne[:], axis=AX.X, op=OP.add)
            nc.sync.dma_start(out=out_e[:], in_=res[:])

    nc.compile()
    return nc


def _wrap16(blk):
    w = blk.reshape(-1, 16).T
    return np.tile(w, (8, 1)).astype(np.int16)


def _plane(arr, dtype=np.float32):
    return np.ascontiguousarray(arr.reshape(-1, P).T).astype(dtype)


def _deal(blk_ids, nblk, ncores, cap_round=1):
    """Deal items to cores per block, round-robin. Returns (per-core position
    lists, caps). Items of block b on core c land at a common offset table."""
    order = np.argsort(blk_ids, kind="stable")
    counts = np.bincount(blk_ids, minlength=nblk)
    caps = (counts + ncores - 1) // ncores
    return order, counts, caps


def kernel(beta, z0, v0, a0, u, v, event_times, pair_u, pair_v, t0, tn):
    assert not np.any(np.asarray(a0)), "kernel assumes a0 == 0"
    beta = np.asarray(beta, np.float32)
    z0 = np.asarray(z0, np.float32)
    v0 = np.asarray(v0, np.float32)
    u = np.asarray(u).astype(np.int64)
    v = np.asarray(v).astype(np.int64)
    event_times = np.asarray(event_times, np.float32)
    pair_u = np.asarray(pair_u).astype(np.int64)
    pair_v = np.asarray(pair_v).astype(np.int64)
    t0f = float(np.asarray(t0))
    tnf = float(np.asarray(tn))
    b = float(beta.reshape(-1)[0])
    E = u.shape[0]
    NPAIR = pair_u.shape[0]

    # ---- deal events/pairs to cores by u-block ----
    ublk = (u // P).astype(np.int64)
    order, counts, caps = _deal(ublk, NBLK128, N_CORES)
    assert caps.sum() <= NEV_PAD, caps.sum()
    pblk = (pair_u // P).astype(np.int64)
    porder, pcounts, pcaps = _deal(pblk, NBLK128, N_CORES)
    assert pcaps.sum() <= NPR_PAD, pcaps.sum()

    ev_segs = _segments(caps, NEV_PAD)
    pr_segs = _segments(pcaps, NPR_PAD)
    key = (tuple(caps), tuple(pcaps))
    if key not in _cache:
        _cache[key] = _build(ev_segs, pr_segs)
    nc = _cache[key]

    # per-core slot assignment: block b's events at cols [off_b, off_b+cap_b)
    offs = np.zeros(NBLK128 + 1, np.int64)
    offs[1:] = np.cumsum(caps)
    poffs = np.zeros(NBLK128 + 1, np.int64)
    poffs[1:] = np.cumsum(pcaps)

    # positions: sorted events of block b occupy order[cstart_b : cstart_b+counts_b];
    # deal index j -> core j % 8, slot off_b + j // 8
    cstart = np.zeros(NBLK128 + 1, np.int64)
    cstart[1:] = np.cumsum(counts)
    pcstart = np.zeros(NBLK128 + 1, np.int64)
    pcstart[1:] = np.cumsum(pcounts)

    def placements(order_, counts_, cstart_, offs_, npad):
        """-> core id and padded position for each original item (by sorted order)."""
        n = order_.shape[0]
        j_in_blk = np.arange(n) - np.repeat(cstart_[:-1], counts_)
        cores = j_in_blk % N_CORES
        slots = np.repeat(offs_[:-1], counts_) + j_in_blk // N_CORES
        return cores, slots  # aligned with order_

    ev_cores, ev_slots = placements(order, counts, cstart, offs, NEV_PAD)
    pr_cores, pr_slots = placements(porder, pcounts, pcstart, poffs, NPR_PAD)

    # ---- tables ----
    tblb = np.zeros((P, NBLK128 * 4), np.float32)
    n = np.arange(N_POINTS)
    tblb[n % P, 4 * (n // P) + 0] = z0[:, 0]
    tblb[n % P, 4 * (n // P) + 1] = z0[:, 1]
    tblb[n % P, 4 * (n // P) + 2] = v0[:, 0]
    tblb[n % P, 4 * (n // P) + 3] = v0[:, 1]
    tblb = tblb.astype(ml_dtypes.bfloat16)

    tb32 = np.zeros((NBLK4 * 4, 16), np.float32)
    tb32[:N_POINTS, 0:2] = z0
    tb32[:N_POINTS, 2:4] = v0
    tb32 = np.ascontiguousarray(tb32.reshape(NBLK4, 64))

    dt = (tnf - t0f) / N_RIEMANN
    ts = (t0f + (np.arange(N_RIEMANN, dtype=np.float32) / N_RIEMANN)
          * (tnf - t0f)).astype(np.float32)
    t2d = np.tile(ts[None, :], (P, 1))

    in_maps = []
    for c in range(N_CORES):
        sel = ev_cores == c
        so = order[sel]              # original event ids for this core
        slots = ev_slots[sel]
        uu = np.zeros(NEV_PAD, np.int64)
        vv = np.full(NEV_PAD, 4 * (NBLK4 - 1), np.int64)  # pad -> zero block
        tt = np.zeros(NEV_PAD, np.float32)
        live = np.zeros(NEV_PAD, bool)
        uu[slots] = u[so]
        vv[slots] = v[so]
        tt[slots] = event_times[so]
        live[slots] = True
        ohu = np.zeros((P, NEV_PAD), ml_dtypes.float8_e4m3fn)
        ohu[uu[live] % P, np.flatnonzero(live)] = 1.0
        mskf = np.zeros((4, NEV_PAD), np.float32)
        mskf[vv[live] % 4, np.flatnonzero(live)] = 1.0
        msk = np.stack([_plane(mskf[s], np.int8) for s in range(4)], axis=1)

        psel = pr_cores == c
        pso = porder[psel]
        pslots = pr_slots[psel]
        pu_ = np.zeros(NPR_PAD, np.int64)
        pv_ = np.full(NPR_PAD, 4 * (NBLK4 - 1), np.int64)
        plive = np.zeros(NPR_PAD, bool)
        pu_[pslots] = pair_u[pso]
        pv_[pslots] = pair_v[pso]
        plive[pslots] = True
        ohpr = np.zeros((P, NPR_PAD), ml_dtypes.float8_e4m3fn)
        ohpr[pu_[plive] % P, np.flatnonzero(plive)] = 1.0
        pmskf = np.zeros((4, NPR_PAD), np.float32)
        pmskf[pv_[plive] % 4, np.flatnonzero(plive)] = 1.0
        pmsk = np.stack([_plane(pmskf[s], np.int8) for s in range(4)], axis=1)

        in_maps.append({
            "tblb": tblb,
            "ohu": ohu,
            "vbl": _wrap16(vv // 4),
            "te": _plane(tt),
            "msk": msk,
            "table32": tb32,
            "ohp": ohpr,
            "pvb": _wrap16(pv_ // 4),
            "pmsk": pmsk,
            "t2d": t2d,
            "t2g": np.tile(ts[None, None, :], (P, 13, 1)),
            "betac": np.full((1, 1), b, np.float32),
        })

    trace = bool(_os.environ.get("KERNEL_TRACE"))
    if trace:
        try:
            import types
            if "antenv.axon_hooks" not in _sys.modules:
                mod = types.ModuleType("antenv.axon_hooks")
                mod._hook = None
                mod.set_axon_ntff_profile_hook = lambda h: setattr(mod, "_hook", h)
                mod.get_axon_ntff_profile_hook = lambda: mod._hook
                import antenv
                antenv.axon_hooks = mod
                _sys.modules["antenv.axon_hooks"] = mod
                from trn_agent_boot.trn_boot import _ntff_profile_via_ctypes
                hk = _ntff_profile_via_ctypes("/opt/axon/libaxon_pjrt.so")
                if hk is not None:
                    mod.set_axon_ntff_profile_hook(hk)
        except Exception:
            trace = False
    from concourse.bass_utils import run_bass_kernel_spmd
    r = run_bass_kernel_spmd(nc, in_maps, core_ids=list(range(N_CORES)),
                             trace=trace)
    globals()["LAST_EXEC_NS"] = r.exec_time_ns

    ev_sum = 0.0
    ne_sum = 0.0
    for c in range(N_CORES):
        out = r.results[c]["out"].astype(np.float64)
        ev_sum += out[:, 0].sum()
        ne_sum += out[:, 1].sum()

    # pad corrections: event pad -> d = sqrt(2)*eps; pair pad -> per t step
    d_dummy = np.sqrt(2.0) * EPS
    n_ev_dummy = N_CORES * NEV_PAD - E
    n_pr_dummy = N_CORES * NPR_PAD - NPAIR
    ev_sum -= n_ev_dummy * d_dummy
    ne_sum -= n_pr_dummy * N_RIEMANN * np.exp(b - d_dummy)

    globals()["DEBUG_PARTS"] = (ev_sum, ne_sum)
    result = b * E - ev_sum - NON_EVENT_W * ne_sum * dt
    return np.float32(result)


# revision 10
# speedup vs baseline: 1.9619x; 1.9365x over previous
"""Trainium2 Bass kernel for BasicEuclideanDistModel log-likelihood (v2).

result = beta*E - sum_e ||dz_uv + dv_uv*t_e + eps|| - dt*sum_{p,j} exp(beta - d_p(t_j))

Architecture (8 cores, SPMD):
- Events dealt to cores round-robin per 128-node u-block so every core has an
  identical per-block count (cap_b); fixed segment structure -> one graph.
- u-side of events/pairs: TensorEngine one-hot gather. Stationary = bf16 table
  block [128 nodes, 4 feats]; moving = host-built fp8 one-hot [128, N].
  PSUM [4, 512] tiles stacked at partition bases {0,32,64}, PE-transposed to
  [128, C] feature planes.
- v-side: SWDGE dma_gather of 256B 4-node blocks (f32), 8192-idx batches on 4
  queues, slot extracted with 4 host mask planes.
- Event d: ||(zu-zv) + (vu-vv)t + eps||; pairs: d^2 = A + t(B + tC) quadratic
  form, exp(beta-d) accumulated on ACT engine.
- Per-core partials [128, 2]; host sums + subtracts pad-dummy contributions.
"""
import os as _os
import sys as _sys
import numpy as np
import ml_dtypes

N_POINTS = 100000
N_RIEMANN = 128
EPS = 1e-6
NON_EVENT_W = 1.0
N_CORES = 8
P = 128
NBLK128 = 782          # 128-node blocks for PE gather
NBLK4 = 25001          # 4-node blocks (+1 zero pad block) for SWDGE
NEV_PAD = 131072       # events per core, padded (16 chunks x 8192)
NCH = 16
CHUNK = 8192
NPR_PAD = 13312        # pairs per core, padded (104 cols)

_cache = {}


def _segments(caps, total):
    """Fixed segment list from per-block capacities.

    Returns list of (c0, c1, blk) covering [0, total), split at block
    boundaries and 512-column PT-tile boundaries. Tail beyond sum(caps)
    is covered with blk=0 (zero one-hots there).
    """
    bounds = []
    off = 0
    for b, cap in enumerate(caps):
        if cap:
            bounds.append((off, off + cap, b))
            off += cap
    assert off <= total
    if off < total:
        bounds.append((off, total, 0))
    segs = []
    for (c0, c1, b) in bounds:
        c = c0
        while c < c1:
            nxt = min(c1, (c // 512 + 1) * 512)
            segs.append((c, nxt, b))
            c = nxt
    return segs


def _build(ev_segs, pr_segs):
    import concourse.bacc as bacc
    import concourse.mybir as mybir
    import concourse.tile as tile
    from concourse import masks as cmasks

    f32 = mybir.dt.float32
    bf16 = mybir.dt.bfloat16
    f8 = mybir.dt.float8e4
    i16 = mybir.dt.int16
    i8 = mybir.dt.int8
    AX = mybir.AxisListType
    OP = mybir.AluOpType
    ACT = mybir.ActivationFunctionType

    nc = bacc.Bacc(num_swdge_queues=4, dynamic_dma_scratch_size=16384)
    tblb_e = nc.declare_dram_parameter("tblb", [P, NBLK128 * 4], bf16, isOutput=False)
    ohu_e = nc.declare_dram_parameter("ohu", [P, NEV_PAD], f8, isOutput=False)
    vbl_e = nc.declare_dram_parameter("vbl", [P, NEV_PAD // 16], i16, isOutput=False)
    te_e = nc.declare_dram_parameter("te", [P, NEV_PAD // P], f32, isOutput=False)
    msk_e = nc.declare_dram_parameter("msk", [P, 4, NEV_PAD // P], i8, isOutput=False)
    tb32_e = nc.declare_dram_parameter("table32", [NBLK4, 64], f32, isOutput=False)
    ohp_e = nc.declare_dram_parameter("ohp", [P, NPR_PAD], f8, isOutput=False)
    pvb_e = nc.declare_dram_parameter("pvb", [P, NPR_PAD // 16], i16, isOutput=False)
    pmsk_e = nc.declare_dram_parameter("pmsk", [P, 4, NPR_PAD // P], i8, isOutput=False)
    t2d_e = nc.declare_dram_parameter("t2d", [P, N_RIEMANN], f32, isOutput=False)
    t2g_e = nc.declare_dram_parameter("t2g", [P, 13, N_RIEMANN], f32, isOutput=False)
    bt_e = nc.declare_dram_parameter("betac", [1, 1], f32, isOutput=False)
    out_e = nc.declare_dram_parameter("out", [P, 2], f32, isOutput=True)

    PRC = NPR_PAD // P      # 100 pair cols
    ECC = CHUNK // P        # 128 event cols per chunk

    # segment lists grouped by PT tile index
    def segs_by_tile(segs):
        by = {}
        for s in segs:
            by.setdefault(s[0] // 512, []).append(s)
        return by

    ev_by = segs_by_tile(ev_segs)
    pr_by = segs_by_tile(pr_segs)

    with tile.TileContext(nc) as tc:
        with tc.tile_pool(name="persist", bufs=1) as pp, \
             tc.tile_pool(name="oh", bufs=3) as ohp, \
             tc.tile_pool(name="gv", bufs=4) as gvp, \
             tc.tile_pool(name="wk", bufs=2) as wk, \
             tc.tile_pool(name="wp", bufs=2) as wp, \
             tc.tile_pool(name="bk", bufs=2, space="PSUM") as bkp, \
             tc.tile_pool(name="tp", bufs=2, space="PSUM") as tpp:
            tblb = pp.tile([P, NBLK128 * 4], bf16)
            nc.sync.dma_start(out=tblb[:], in_=tblb_e[:])
            te = pp.tile([P, NEV_PAD // P], f32)
            nc.sync.dma_start(out=te[:], in_=te_e[:])
            msk = pp.tile([P, 4, NEV_PAD // P], i8)
            nc.sync.dma_start(out=msk[:], in_=msk_e[:])
            vbl = pp.tile([P, NEV_PAD // 16], i16)
            nc.sync.dma_start(out=vbl[:], in_=vbl_e[:])
            ohpr = pp.tile([P, NPR_PAD], f8)
            nc.sync.dma_start(out=ohpr[:], in_=ohp_e[:])
            pvb = pp.tile([P, NPR_PAD // 16], i16)
            nc.sync.dma_start(out=pvb[:], in_=pvb_e[:])
            pmsk = pp.tile([P, 4, NPR_PAD // P], i8)
            nc.sync.dma_start(out=pmsk[:], in_=pmsk_e[:])
            t2d = pp.tile([P, N_RIEMANN], f32)
            nc.sync.dma_start(out=t2d[:], in_=t2d_e[:])
            t2g = pp.tile([P, 13, N_RIEMANN], f32)
            nc.sync.dma_start(out=t2g[:], in_=t2g_e[:])
            bt1 = pp.tile([1, 1], f32)
            nc.sync.dma_start(out=bt1[:], in_=bt_e[:])
            bcol = pp.tile([P, 1], f32)
            nc.gpsimd.partition_broadcast(bcol[:], bt1[:])
            epsc = pp.tile([P, 1], f32)
            nc.vector.memset(epsc[:], EPS)
            ident = pp.tile([P, P], f32)
            cmasks.make_identity(nc, ident[:])

            acc_ev = pp.tile([P, NCH], f32)
            acc_ne = pp.tile([P, PRC // 13], f32)

            # ---------- PE one-hot gather -> feature planes ----------
            def pe_gather(oh_tile, by_tile, tiles0, ntiles, ftile):
                """Gather ntiles PT tiles (512 ev each) from global PT index
                tiles0 into ftile [128, 4*ntiles cols, 4 feats]."""
                g = 0
                while g * 3 < ntiles:
                    tl = [3 * g + i for i in range(3) if 3 * g + i < ntiles]
                    bank = bkp.tile([P, 512], f32, tag="bank")
                    for bi, t in enumerate(tl):
                        for (c0, c1, b) in by_tile.get(tiles0 + t, []):
                            lc0 = c0 - (tiles0 + t) * 512
                            oc0 = c0 - tiles0 * 512
                            nc.tensor.matmul(
                                out=bank[32 * bi:32 * bi + 4, lc0:lc0 + (c1 - c0)],
                                lhsT=tblb[:, 4 * b:4 * b + 4],
                                rhs=oh_tile[:, oc0:oc0 + (c1 - c0)],
                                start=True, stop=True)
                    stack = wk.tile([P, 512], f32, tag="stack")
                    nc.vector.tensor_copy(out=stack[:], in_=bank[:])
                    tb = tpp.tile([P, 4, 4, 32], f32, tag="tb")
                    for j in range(4):
                        nc.tensor.transpose(out=tb[:, j], in_=stack[:, P * j:P * j + P],
                                            identity=ident[:])
                    for bi, t in enumerate(tl):
                        nc.vector.tensor_copy(
                            out=ftile[:, 4 * t:4 * t + 4, :],
                            in_=tb[:, :, bi, 0:4])
                    g += 1

            # ---------- events ----------
            for c in range(NCH):
                ohu_t = ohp.tile([P, CHUNK], f8, tag="ohu")
                nc.sync.dma_start(out=ohu_t[:], in_=ohu_e[:, c * CHUNK:(c + 1) * CHUNK])
                gv = gvp.tile([P, ECC, 64], f32, tag="gv")
                for qi in range(4):
                    q0 = c * CHUNK + qi * (CHUNK // 4)
                    nc.gpsimd.dma_gather(
                        out_ap=gv[:, qi * (ECC // 4):(qi + 1) * (ECC // 4), :],
                        in_ap=tb32_e[:],
                        idxs_ap=vbl[:, q0 // 16:(q0 + CHUNK // 4) // 16],
                        num_idxs=CHUNK // 4, num_idxs_reg=CHUNK // 4, elem_size=64,
                        single_packet=False, queue_num=qi)
                F = wk.tile([P, ECC, 4], f32, tag="F")
                pe_gather(ohu_t, ev_by, c * (CHUNK // 512), CHUNK // 512, F)
                # v-side slot select + d compute
                mc = msk[:, :, c * ECC:(c + 1) * ECC]
                tec = te[:, c * ECC:(c + 1) * ECC]
                dd = []
                for f in range(4):
                    sel = wp.tile([P, ECC], f32, tag=f"sel{f}", name=f"sel{f}")
                    nc.vector.tensor_copy(out=sel[:], in_=gv[:, :, f])
                    for s in range(1, 4):
                        nc.vector.copy_predicated(sel[:], mc[:, s],
                                                  gv[:, :, 16 * s + f])
                    d = wp.tile([P, ECC], f32, tag=f"d{f}", name=f"d{f}")
                    nc.vector.tensor_tensor(out=d[:], in0=F[:, :, f], in1=sel[:],
                                            op=OP.subtract)
                    dd.append(d)
                dzx, dzy, dvx, dvy = dd
                ws = wp.tile([P, 2, ECC], f32, tag="ws")
                nc.vector.tensor_tensor(out=ws[:, 0], in0=dvx[:], in1=tec, op=OP.mult)
                nc.vector.tensor_tensor(out=ws[:, 0], in0=ws[:, 0], in1=dzx[:],
                                        op=OP.add)
                nc.vector.tensor_tensor(out=ws[:, 1], in0=dvy[:], in1=tec, op=OP.mult)
                nc.vector.tensor_tensor(out=ws[:, 1], in0=ws[:, 1], in1=dzy[:],
                                        op=OP.add)
                sq = wp.tile([P, 2, ECC], f32, tag="sq")
                nc.scalar.activation(sq[:], ws[:], ACT.Square, bias=epsc[:])
                nc.vector.tensor_tensor(out=sq[:, 0], in0=sq[:, 0], in1=sq[:, 1],
                                        op=OP.add)
                dt_ = wp.tile([P, ECC], f32, tag="dt")
                nc.scalar.activation(dt_[:], sq[:, 0], ACT.Sqrt,
                                     accum_out=acc_ev[:, c:c + 1])

            # ---------- pairs ----------
            pF = pp.tile([P, PRC, 4], f32)
            pe_gather(ohpr, pr_by, 0, NPR_PAD // 512, pF)
            pd = [pp.tile([P, PRC], f32, name=f"pd{f}") for f in range(4)]
            for h in range(2):
                gpv = gvp.tile([P, 64, 64], f32, tag="gv", name="gpv")
                for qi in range(4):
                    q0 = h * 6656 + qi * 1664
                    nc.gpsimd.dma_gather(
                        out_ap=gpv[:, qi * 13:(qi + 1) * 13, :], in_ap=tb32_e[:],
                        idxs_ap=pvb[:, q0 // 16:(q0 + 1664) // 16],
                        num_idxs=1664, num_idxs_reg=1664, elem_size=64,
                        single_packet=False, queue_num=qi)
                hs = slice(52 * h, 52 * (h + 1))
                for f in range(4):
                    sel = wp.tile([P, 52], f32, tag="psel")
                    nc.vector.tensor_copy(out=sel[:], in_=gpv[:, 0:52, f])
                    for s in range(1, 4):
                        nc.vector.copy_predicated(sel[:], pmsk[:, s, hs],
                                                  gpv[:, 0:52, 16 * s + f])
                    nc.vector.tensor_tensor(out=pd[f][:, hs], in0=pF[:, hs, f],
                                            in1=sel[:], op=OP.subtract)
            dzx, dzy, dvx, dvy = pd
            # A = (dz+eps)^2 sum; B = 2 (dz+eps).dv; C = dv.dv
            px = pp.tile([P, PRC], f32)
            nc.vector.tensor_tensor(out=px[:], in0=dzx[:],
                                    in1=epsc[:].to_broadcast(dzx.shape), op=OP.add)
            py = pp.tile([P, PRC], f32)
            nc.vector.tensor_tensor(out=py[:], in0=dzy[:],
                                    in1=epsc[:].to_broadcast(dzy.shape), op=OP.add)
            At = pp.tile([P, PRC, 1], f32)
            Bt = pp.tile([P, PRC, 1], f32)
            Ct = pp.tile([P, PRC, 1], f32)
            t1 = wp.tile([P, PRC], f32, tag="pt1")
            nc.vector.tensor_tensor(out=At[:, :, 0], in0=px[:], in1=px[:], op=OP.mult)
            nc.vector.tensor_tensor(out=t1[:], in0=py[:], in1=py[:], op=OP.mult)
            nc.vector.tensor_tensor(out=At[:, :, 0], in0=At[:, :, 0], in1=t1[:],
                                    op=OP.add)
            nc.vector.tensor_tensor(out=Bt[:, :, 0], in0=px[:], in1=dvx[:], op=OP.mult)
            nc.vector.tensor_tensor(out=t1[:], in0=py[:], in1=dvy[:], op=OP.mult)
            nc.vector.tensor_tensor(out=Bt[:, :, 0], in0=Bt[:, :, 0], in1=t1[:],
                                    op=OP.add)
            nc.vector.tensor_tensor(out=Bt[:, :, 0], in0=Bt[:, :, 0], in1=Bt[:, :, 0],
                                    op=OP.add)
            nc.vector.tensor_tensor(out=Ct[:, :, 0], in0=dvx[:], in1=dvx[:], op=OP.mult)
            nc.vector.tensor_tensor(out=t1[:], in0=dvy[:], in1=dvy[:], op=OP.mult)
            nc.vector.tensor_tensor(out=Ct[:, :, 0], in0=Ct[:, :, 0], in1=t1[:],
                                    op=OP.add)
            GW = 13
            for q in range(PRC // GW):
                ks = slice(q * GW, (q + 1) * GW)
                shp = [P, GW, N_RIEMANN]
                w1 = wp.tile(shp, f32, tag="w1", bufs=1)
                nc.vector.tensor_tensor(out=w1[:], in0=t2g[:],
                                        in1=Ct[:, ks, :].to_broadcast(shp),
                                        op=OP.mult)
                nc.vector.tensor_tensor(out=w1[:], in0=w1[:],
                                        in1=Bt[:, ks, :].to_broadcast(shp),
                                        op=OP.add)
                nc.vector.tensor_tensor(out=w1[:], in0=w1[:], in1=t2g[:], op=OP.mult)
                nc.vector.tensor_tensor(out=w1[:], in0=w1[:],
                                        in1=At[:, ks, :].to_broadcast(shp),
                                        op=OP.add)
                w2 = wp.tile(shp, f32, tag="w2", bufs=1)
                nc.scalar.activation(w2[:], w1[:], ACT.Sqrt)
                w3 = wp.tile(shp, f32, tag="w3", bufs=1)
                nc.scalar.activation(w3[:], w2[:], ACT.Exp, bias=bcol[:],
                                     scale=-1.0, accum_out=acc_ne[:, q:q + 1])

            res = pp.tile([P, 2], f32)
            nc.vector.tensor_reduce(res[:, 0:1], acc_ev[:], axis=AX.X, op=OP.add)
            nc.vector.tensor_reduce(res[:, 1:2], acc_ne[:], axis=AX.X, op=OP.add)
            nc.sync.dma_start(out=out_e[:], in_=res[:])

    nc.compile()
    return nc


def _wrap16(blk):
    w = blk.reshape(-1, 16).T
    return np.tile(w, (8, 1)).astype(np.int16)


def _plane(arr, dtype=np.float32):
    return np.ascontiguousarray(arr.reshape(-1, P).T).astype(dtype)


def _deal(blk_ids, nblk, ncores, cap_round=1):
    """Deal items to cores per block, round-robin. Returns (per-core position
    lists, caps). Items of block b on core c land at a common offset table."""
    order = np.argsort(blk_ids, kind="stable")
    counts = np.bincount(blk_ids, minlength=nblk)
    caps = (counts + ncores - 1) // ncores
    return order, counts, caps


def kernel(beta, z0, v0, a0, u, v, event_times, pair_u, pair_v, t0, tn):
    assert not np.any(np.asarray(a0)), "kernel assumes a0 == 0"
    beta = np.asarray(beta, np.float32)
    z0 = np.asarray(z0, np.float32)
    v0 = np.asarray(v0, np.float32)
    u = np.asarray(u).astype(np.int64)
    v = np.asarray(v).astype(np.int64)
    event_times = np.asarray(event_times, np.float32)
    pair_u = np.asarray(pair_u).astype(np.int64)
    pair_v = np.asarray(pair_v).astype(np.int64)
    t0f = float(np.asarray(t0))
    tnf = float(np.asarray(tn))
    b = float(beta.reshape(-1)[0])
    E = u.shape[0]
    NPAIR = pair_u.shape[0]

    # ---- deal events/pairs to cores by u-block ----
    ublk = (u // P).astype(np.int64)
    order, counts, caps = _deal(ublk, NBLK128, N_CORES)
    assert caps.sum() <= NEV_PAD, caps.sum()
    pblk = (pair_u // P).astype(np.int64)
    porder, pcounts, pcaps = _deal(pblk, NBLK128, N_CORES)
    assert pcaps.sum() <= NPR_PAD, pcaps.sum()

    ev_segs = _segments(caps, NEV_PAD)
    pr_segs = _segments(pcaps, NPR_PAD)
    key = (tuple(caps), tuple(pcaps))
    if key not in _cache:
        _cache[key] = _build(ev_segs, pr_segs)
    nc = _cache[key]

    # per-core slot assignment: block b's events at cols [off_b, off_b+cap_b)
    offs = np.zeros(NBLK128 + 1, np.int64)
    offs[1:] = np.cumsum(caps)
    poffs = np.zeros(NBLK128 + 1, np.int64)
    poffs[1:] = np.cumsum(pcaps)

    # positions: sorted events of block b occupy order[cstart_b : cstart_b+counts_b];
    # deal index j -> core j % 8, slot off_b + j // 8
    cstart = np.zeros(NBLK128 + 1, np.int64)
    cstart[1:] = np.cumsum(counts)
    pcstart = np.zeros(NBLK128 + 1, np.int64)
    pcstart[1:] = np.cumsum(pcounts)

    def placements(order_, counts_, cstart_, offs_, npad):
        """-> core id and padded position for each original item (by sorted order)."""
        n = order_.shape[0]
        j_in_blk = np.arange(n) - np.repeat(cstart_[:-1], counts_)
        cores = j_in_blk % N_CORES
        slots = np.repeat(offs_[:-1], counts_) + j_in_blk // N_CORES
        return cores, slots  # aligned with order_

    ev_cores, ev_slots = placements(order, counts, cstart, offs, NEV_PAD)
    pr_cores, pr_slots = placements(porder, pcounts, pcstart, poffs, NPR_PAD)

    # ---- tables ----
    tblb = np.zeros((P, NBLK128 * 4), np.float32)
    n = np.arange(N_POINTS)
    tblb[n % P, 4 * (n // P) + 0] = z0[:, 0]
    tblb[n % P, 4 * (n // P) + 1] = z0[:, 1]
    tblb[n % P, 4 * (n // P) + 2] = v0[:, 0]
    tblb[n % P, 4 * (n // P) + 3] = v0[:, 1]
    tblb = tblb.astype(ml_dtypes.bfloat16)

    tb32 = np.zeros((NBLK4 * 4, 16), np.float32)
    tb32[:N_POINTS, 0:2] = z0
    tb32[:N_POINTS, 2:4] = v0
    tb32 = np.ascontiguousarray(tb32.reshape(NBLK4, 64))

    dt = (tnf - t0f) / N_RIEMANN
    ts = (t0f + (np.arange(N_RIEMANN, dtype=np.float32) / N_RIEMANN)
          * (tnf - t0f)).astype(np.float32)
    t2d = np.tile(ts[None, :], (P, 1))

    in_maps = []
    for c in range(N_CORES):
        sel = ev_cores == c
        so = order[sel]              # original event ids for this core
        slots = ev_slots[sel]
        uu = np.zeros(NEV_PAD, np.int64)
        vv = np.full(NEV_PAD, 4 * (NBLK4 - 1), np.int64)  # pad -> zero block
        tt = np.zeros(NEV_PAD, np.float32)
        live = np.zeros(NEV_PAD, bool)
        uu[slots] = u[so]
        vv[slots] = v[so]
        tt[slots] = event_times[so]
        live[slots] = True
        ohu = np.zeros((P, NEV_PAD), ml_dtypes.float8_e4m3fn)
        ohu[uu[live] % P, np.flatnonzero(live)] = 1.0
        mskf = np.zeros((4, NEV_PAD), np.float32)
        mskf[vv[live] % 4, np.flatnonzero(live)] = 1.0
        msk = np.stack([_plane(mskf[s], np.int8) for s in range(4)], axis=1)

        psel = pr_cores == c
        pso = porder[psel]
        pslots = pr_slots[psel]
        pu_ = np.zeros(NPR_PAD, np.int64)
        pv_ = np.full(NPR_PAD, 4 * (NBLK4 - 1), np.int64)
        plive = np.zeros(NPR_PAD, bool)
        pu_[pslots] = pair_u[pso]
        pv_[pslots] = pair_v[pso]
        plive[pslots] = True
        ohpr = np.zeros((P, NPR_PAD), ml_dtypes.float8_e4m3fn)
        ohpr[pu_[plive] % P, np.flatnonzero(plive)] = 1.0
        pmskf = np.zeros((4, NPR_PAD), np.float32)
        pmskf[pv_[plive] % 4, np.flatnonzero(plive)] = 1.0
        pmsk = np.stack([_plane(pmskf[s], np.int8) for s in range(4)], axis=1)

        in_maps.append({
            "tblb": tblb,
            "ohu": ohu,
            "vbl": _wrap16(vv // 4),
            "te": _plane(tt),
            "msk": msk,
            "table32": tb32,
            "ohp": ohpr,
            "pvb": _wrap16(pv_ // 4),
            "pmsk": pmsk,
            "t2d": t2d,
            "t2g": np.tile(ts[None, None, :], (P, 13, 1)),
            "betac": np.full((1, 1), b, np.float32),
        })

    trace = bool(_os.environ.get("KERNEL_TRACE"))
    if trace:
        try:
            import types
            if "antenv.axon_hooks" not in _sys.modules:
                mod = types.ModuleType("antenv.axon_hooks")
                mod._hook = None
                mod.set_axon_ntff_profile_hook = lambda h: setattr(mod, "_hook", h)
                mod.get_axon_ntff_profile_hook = lambda: mod._hook
                import antenv
                antenv.axon_hooks = mod
                _sys.modules["antenv.axon_hooks"] = mod
                from trn_agent_boot.trn_boot import _ntff_profile_via_ctypes
                hk = _ntff_profile_via_ctypes("/opt/axon/libaxon_pjrt.so")
                if hk is not None:
                    mod.set_axon_ntff_profile_hook(hk)
        except Exception:
            trace = False
    from concourse.bass_utils import run_bass_kernel_spmd
    r = run_bass_kernel_spmd(nc, in_maps, core_ids=list(range(N_CORES)),
                             trace=trace)
    globals()["LAST_EXEC_NS"] = r.exec_time_ns

    ev_sum = 0.0
    ne_sum = 0.0
    for c in range(N_CORES):
        out = r.results[c]["out"].astype(np.float64)
        ev_sum += out[:, 0].sum()
        ne_sum += out[:, 1].sum()

    # pad corrections: event pad -> d = sqrt(2)*eps; pair pad -> per t step
    d_dummy = np.sqrt(2.0) * EPS
    n_ev_dummy = N_CORES * NEV_PAD - E
    n_pr_dummy = N_CORES * NPR_PAD - NPAIR
    ev_sum -= n_ev_dummy * d_dummy
    ne_sum -= n_pr_dummy * N_RIEMANN * np.exp(b - d_dummy)

    globals()["DEBUG_PARTS"] = (ev_sum, ne_sum)
    result = b * E - ev_sum - NON_EVENT_W * ne_sum * dt
    return np.float32(result)


# revision 13
# speedup vs baseline: 2.0802x; 1.0603x over previous
"""Trainium2 Bass kernel for BasicEuclideanDistModel log-likelihood (v2).

result = beta*E - sum_e ||dz_uv + dv_uv*t_e + eps|| - dt*sum_{p,j} exp(beta - d_p(t_j))

Architecture (8 cores, SPMD):
- Events dealt to cores round-robin per 128-node u-block so every core has an
  identical per-block count (cap_b); fixed segment structure -> one graph.
- u-side of events/pairs: TensorEngine one-hot gather. Stationary = bf16 table
  block [128 nodes, 4 feats]; moving = host-built fp8 one-hot [128, N].
  PSUM [4, 512] tiles stacked at partition bases {0,32,64}, PE-transposed to
  [128, C] feature planes.
- v-side: SWDGE dma_gather of 256B 4-node blocks (f32), 8192-idx batches on 4
  queues, slot extracted with 4 host mask planes.
- Event d: ||(zu-zv) + (vu-vv)t + eps||; pairs: d^2 = A + t(B + tC) quadratic
  form, exp(beta-d) accumulated on ACT engine.
- Per-core partials [128, 2]; host sums + subtracts pad-dummy contributions.
"""
import os as _os
import sys as _sys
import numpy as np
import ml_dtypes

N_POINTS = 100000
N_RIEMANN = 128
EPS = 1e-6
NON_EVENT_W = 1.0
N_CORES = 8
P = 128
NBLK128 = 782          # 128-node blocks for PE gather
NBLK4 = 25001          # 4-node blocks (+1 zero pad block) for SWDGE
NEV_PAD = 131072       # events per core, padded (16 chunks x 8192)
NCH = 16
CHUNK = 8192
NPR_PAD = 13312        # pairs per core, padded (104 cols)

_cache = {}


def _segments(caps, total):
    """Fixed segment list from per-block capacities.

    Returns list of (c0, c1, blk) covering [0, total), split at block
    boundaries and 512-column PT-tile boundaries. Tail beyond sum(caps)
    is covered with blk=0 (zero one-hots there).
    """
    bounds = []
    off = 0
    for b, cap in enumerate(caps):
        if cap:
            bounds.append((off, off + cap, b))
            off += cap
    assert off <= total
    if off < total:
        bounds.append((off, total, 0))
    segs = []
    for (c0, c1, b) in bounds:
        c = c0
        while c < c1:
            nxt = min(c1, (c // 512 + 1) * 512)
            segs.append((c, nxt, b))
            c = nxt
    return segs


def _build(ev_segs, pr_segs):
    import concourse.bacc as bacc
    import concourse.mybir as mybir
    import concourse.tile as tile
    from concourse import masks as cmasks

    f32 = mybir.dt.float32
    bf16 = mybir.dt.bfloat16
    f8 = mybir.dt.float8e4
    i16 = mybir.dt.int16
    i8 = mybir.dt.int8
    AX = mybir.AxisListType
    OP = mybir.AluOpType
    ACT = mybir.ActivationFunctionType

    nc = bacc.Bacc(num_swdge_queues=4, dynamic_dma_scratch_size=16384)
    tblb_e = nc.declare_dram_parameter("tblb", [P, NBLK128 * 4], bf16, isOutput=False)
    ohu_e = nc.declare_dram_parameter("ohu", [P, NEV_PAD], f8, isOutput=False)
    vbl_e = nc.declare_dram_parameter("vbl", [P, NEV_PAD // 16], i16, isOutput=False)
    te_e = nc.declare_dram_parameter("te", [P, NEV_PAD // P], f32, isOutput=False)
    msk_e = nc.declare_dram_parameter("msk", [P, 4, NEV_PAD // P], i8, isOutput=False)
    tb32_e = nc.declare_dram_parameter("table32", [NBLK4, 64], f32, isOutput=False)
    ohp_e = nc.declare_dram_parameter("ohp", [P, NPR_PAD], f8, isOutput=False)
    pvb_e = nc.declare_dram_parameter("pvb", [P, NPR_PAD // 16], i16, isOutput=False)
    pmsk_e = nc.declare_dram_parameter("pmsk", [P, 4, NPR_PAD // P], i8, isOutput=False)
    t2d_e = nc.declare_dram_parameter("t2d", [P, N_RIEMANN], f32, isOutput=False)
    t2g_e = nc.declare_dram_parameter("t2g", [P, 13, N_RIEMANN], f32, isOutput=False)
    bt_e = nc.declare_dram_parameter("betac", [1, 1], f32, isOutput=False)
    out_e = nc.declare_dram_parameter("out", [P, 2], f32, isOutput=True)

    PRC = NPR_PAD // P      # 100 pair cols
    ECC = CHUNK // P        # 128 event cols per chunk

    # segment lists grouped by PT tile index
    def segs_by_tile(segs):
        by = {}
        for s in segs:
            by.setdefault(s[0] // 512, []).append(s)
        return by

    ev_by = segs_by_tile(ev_segs)
    pr_by = segs_by_tile(pr_segs)

    with tile.TileContext(nc) as tc:
        with tc.tile_pool(name="persist", bufs=1) as pp, \
             tc.tile_pool(name="oh", bufs=3) as ohp, \
             tc.tile_pool(name="gv", bufs=4) as gvp, \
             tc.tile_pool(name="wk", bufs=2) as wk, \
             tc.tile_pool(name="wp", bufs=2) as wp, \
             tc.tile_pool(name="bk", bufs=2, space="PSUM") as bkp, \
             tc.tile_pool(name="tp", bufs=2, space="PSUM") as tpp:
            vbl = pp.tile([P, NEV_PAD // 16], i16)
            nc.sync.dma_start(out=vbl[:], in_=vbl_e[:])
            pvb = pp.tile([P, NPR_PAD // 16], i16)
            nc.sync.dma_start(out=pvb[:], in_=pvb_e[:])
            tblb = pp.tile([P, NBLK128 * 4], bf16)
            nc.sync.dma_start(out=tblb[:], in_=tblb_e[:])
            te = pp.tile([P, NEV_PAD // P], f32)
            nc.sync.dma_start(out=te[:], in_=te_e[:])
            msk = pp.tile([P, 4, NEV_PAD // P], i8)
            nc.sync.dma_start(out=msk[:], in_=msk_e[:])
            ohpr = pp.tile([P, NPR_PAD], f8)
            nc.sync.dma_start(out=ohpr[:], in_=ohp_e[:])
            pmsk = pp.tile([P, 4, NPR_PAD // P], i8)
            nc.sync.dma_start(out=pmsk[:], in_=pmsk_e[:])
            t2d = pp.tile([P, N_RIEMANN], f32)
            nc.sync.dma_start(out=t2d[:], in_=t2d_e[:])
            t2g = pp.tile([P, 13, N_RIEMANN], f32)
            nc.sync.dma_start(out=t2g[:], in_=t2g_e[:])
            bt1 = pp.tile([1, 1], f32)
            nc.sync.dma_start(out=bt1[:], in_=bt_e[:])
            bcol = pp.tile([P, 1], f32)
            nc.gpsimd.partition_broadcast(bcol[:], bt1[:])
            epsc = pp.tile([P, 1], f32)
            nc.vector.memset(epsc[:], EPS)
            ident = pp.tile([P, P], f32)
            cmasks.make_identity(nc, ident[:])

            acc_ev = pp.tile([P, NCH], f32)
            acc_ne = pp.tile([P, PRC // 13], f32)

            # ---------- PE one-hot gather -> feature planes ----------
            def pe_gather(oh_tile, by_tile, tiles0, ntiles, ftile, ev_eng=None):
                """Gather ntiles PT tiles (512 ev each) from global PT index
                tiles0 into ftile [128, 4*ntiles cols, 4 feats]."""
                g = 0
                while g * 3 < ntiles:
                    tl = [3 * g + i for i in range(3) if 3 * g + i < ntiles]
                    bank = bkp.tile([P, 512], f32, tag="bank")
                    for bi, t in enumerate(tl):
                        for (c0, c1, b) in by_tile.get(tiles0 + t, []):
                            lc0 = c0 - (tiles0 + t) * 512
                            oc0 = c0 - tiles0 * 512
                            nc.tensor.matmul(
                                out=bank[32 * bi:32 * bi + 4, lc0:lc0 + (c1 - c0)],
                                lhsT=tblb[:, 4 * b:4 * b + 4],
                                rhs=oh_tile[:, oc0:oc0 + (c1 - c0)],
                                start=True, stop=True)
                    stack = wk.tile([P, 512], f32, tag="stack")
                    if ev_eng == "act":
                        nc.scalar.activation(stack[:], bank[:], ACT.Copy)
                    else:
                        nc.vector.tensor_copy(out=stack[:], in_=bank[:])
                    tb = tpp.tile([P, 4, 4, 32], f32, tag="tb")
                    for j in range(4):
                        nc.tensor.transpose(out=tb[:, j], in_=stack[:, P * j:P * j + P],
                                            identity=ident[:])
                    for bi, t in enumerate(tl):
                        nc.vector.tensor_copy(
                            out=ftile[:, 4 * t:4 * t + 4, :],
                            in_=tb[:, :, bi, 0:4])
                    g += 1

            # ---------- pairs ----------
            pF = pp.tile([P, PRC, 4], f32)
            pe_gather(ohpr, pr_by, 0, NPR_PAD // 512, pF)
            pd = [pp.tile([P, PRC], f32, name=f"pd{f}") for f in range(4)]
            for h in range(2):
                gpv = gvp.tile([P, 64, 64], f32, tag="gv", name="gpv")
                for qi in range(4):
                    q0 = h * 6656 + qi * 1664
                    nc.gpsimd.dma_gather(
                        out_ap=gpv[:, qi * 13:(qi + 1) * 13, :], in_ap=tb32_e[:],
                        idxs_ap=pvb[:, q0 // 16:(q0 + 1664) // 16],
                        num_idxs=1664, num_idxs_reg=1664, elem_size=64,
                        single_packet=False, queue_num=qi)
                hs = slice(52 * h, 52 * (h + 1))
                for f in range(4):
                    sel = wp.tile([P, 52], f32, tag="psel")
                    nc.vector.tensor_copy(out=sel[:], in_=gpv[:, 0:52, f])
                    for s in range(1, 4):
                        nc.vector.copy_predicated(sel[:], pmsk[:, s, hs],
                                                  gpv[:, 0:52, 16 * s + f])
                    nc.vector.tensor_tensor(out=pd[f][:, hs], in0=pF[:, hs, f],
                                            in1=sel[:], op=OP.subtract)
            dzx, dzy, dvx, dvy = pd
            # A = (dz+eps)^2 sum; B = 2 (dz+eps).dv; C = dv.dv
            px = pp.tile([P, PRC], f32)
            nc.vector.tensor_tensor(out=px[:], in0=dzx[:],
                                    in1=epsc[:].to_broadcast(dzx.shape), op=OP.add)
            py = pp.tile([P, PRC], f32)
            nc.vector.tensor_tensor(out=py[:], in0=dzy[:],
                                    in1=epsc[:].to_broadcast(dzy.shape), op=OP.add)
            At = pp.tile([P, PRC, 1], f32)
            Bt = pp.tile([P, PRC, 1], f32)
            Ct = pp.tile([P, PRC, 1], f32)
            t1 = wp.tile([P, PRC], f32, tag="pt1")
            nc.vector.tensor_tensor(out=At[:, :, 0], in0=px[:], in1=px[:], op=OP.mult)
            nc.vector.tensor_tensor(out=t1[:], in0=py[:], in1=py[:], op=OP.mult)
            nc.vector.tensor_tensor(out=At[:, :, 0], in0=At[:, :, 0], in1=t1[:],
                                    op=OP.add)
            nc.vector.tensor_tensor(out=Bt[:, :, 0], in0=px[:], in1=dvx[:], op=OP.mult)
            nc.vector.tensor_tensor(out=t1[:], in0=py[:], in1=dvy[:], op=OP.mult)
            nc.vector.tensor_tensor(out=Bt[:, :, 0], in0=Bt[:, :, 0], in1=t1[:],
                                    op=OP.add)
            nc.vector.tensor_tensor(out=Bt[:, :, 0], in0=Bt[:, :, 0], in1=Bt[:, :, 0],
                                    op=OP.add)
            nc.vector.tensor_tensor(out=Ct[:, :, 0], in0=dvx[:], in1=dvx[:], op=OP.mult)
            nc.vector.tensor_tensor(out=t1[:], in0=dvy[:], in1=dvy[:], op=OP.mult)
            nc.vector.tensor_tensor(out=Ct[:, :, 0], in0=Ct[:, :, 0], in1=t1[:],
                                    op=OP.add)
            GW = 13
            for q in range(PRC // GW):
                ks = slice(q * GW, (q + 1) * GW)
                shp = [P, GW, N_RIEMANN]
                w1 = wp.tile(shp, f32, tag="w1", bufs=2)
                nc.vector.tensor_tensor(out=w1[:], in0=t2g[:],
                                        in1=Ct[:, ks, :].to_broadcast(shp),
                                        op=OP.mult)
                nc.vector.tensor_tensor(out=w1[:], in0=w1[:],
                                        in1=Bt[:, ks, :].to_broadcast(shp),
                                        op=OP.add)
                nc.vector.tensor_tensor(out=w1[:], in0=w1[:], in1=t2g[:], op=OP.mult)
                nc.vector.tensor_tensor(out=w1[:], in0=w1[:],
                                        in1=At[:, ks, :].to_broadcast(shp),
                                        op=OP.add)
                w2 = wp.tile(shp, f32, tag="w2", bufs=2)
                nc.scalar.activation(w2[:], w1[:], ACT.Sqrt)
                w3 = wp.tile(shp, f32, tag="w3", bufs=2)
                nc.scalar.activation(w3[:], w2[:], ACT.Exp, bias=bcol[:],
                                     scale=-1.0, accum_out=acc_ne[:, q:q + 1])

            # ---------- events ----------
            for c in range(NCH):
                ohu_t = ohp.tile([P, CHUNK], f8, tag="ohu")
                nc.sync.dma_start(out=ohu_t[:], in_=ohu_e[:, c * CHUNK:(c + 1) * CHUNK])
                gv = gvp.tile([P, ECC, 64], f32, tag="gv")
                for qi in range(4):
                    q0 = c * CHUNK + qi * (CHUNK // 4)
                    nc.gpsimd.dma_gather(
                        out_ap=gv[:, qi * (ECC // 4):(qi + 1) * (ECC // 4), :],
                        in_ap=tb32_e[:],
                        idxs_ap=vbl[:, q0 // 16:(q0 + CHUNK // 4) // 16],
                        num_idxs=CHUNK // 4, num_idxs_reg=CHUNK // 4, elem_size=64,
                        single_packet=False, queue_num=qi)
                F = wk.tile([P, ECC, 4], f32, tag="F")
                pe_gather(ohu_t, ev_by, c * (CHUNK // 512), CHUNK // 512, F,
                          ev_eng="act" if c % 2 else None)
                # v-side slot select + d compute
                mc = msk[:, :, c * ECC:(c + 1) * ECC]
                tec = te[:, c * ECC:(c + 1) * ECC]
                dd = []
                for f in range(4):
                    sel = wp.tile([P, ECC], f32, tag=f"sel{f}", name=f"sel{f}")
                    nc.vector.tensor_copy(out=sel[:], in_=gv[:, :, f])
                    for s in range(1, 4):
                        nc.vector.copy_predicated(sel[:], mc[:, s],
                                                  gv[:, :, 16 * s + f])
                    d = wp.tile([P, ECC], f32, tag=f"d{f}", name=f"d{f}")
                    nc.vector.tensor_tensor(out=d[:], in0=F[:, :, f], in1=sel[:],
                                            op=OP.subtract)
                    dd.append(d)
                dzx, dzy, dvx, dvy = dd
                ws = wp.tile([P, 2, ECC], f32, tag="ws")
                nc.vector.tensor_tensor(out=ws[:, 0], in0=dvx[:], in1=tec, op=OP.mult)
                nc.vector.tensor_tensor(out=ws[:, 0], in0=ws[:, 0], in1=dzx[:],
                                        op=OP.add)
                nc.vector.tensor_tensor(out=ws[:, 1], in0=dvy[:], in1=tec, op=OP.mult)
                nc.vector.tensor_tensor(out=ws[:, 1], in0=ws[:, 1], in1=dzy[:],
                                        op=OP.add)
                sq = wp.tile([P, 2, ECC], f32, tag="sq")
                nc.scalar.activation(sq[:], ws[:], ACT.Square, bias=epsc[:])
                nc.vector.tensor_tensor(out=sq[:, 0], in0=sq[:, 0], in1=sq[:, 1],
                                        op=OP.add)
                dt_ = wp.tile([P, ECC], f32, tag="dt")
                nc.scalar.activation(dt_[:], sq[:, 0], ACT.Sqrt,
                                     accum_out=acc_ev[:, c:c + 1])

            res = pp.tile([P, 2], f32)
            nc.vector.tensor_reduce(res[:, 0:1], acc_ev[:], axis=AX.X, op=OP.add)
            nc.vector.tensor_reduce(res[:, 1:2], acc_ne[:], axis=AX.X, op=OP.add)
            nc.sync.dma_start(out=out_e[:], in_=res[:])

    nc.compile()
    return nc


def _wrap16(blk):
    w = blk.reshape(-1, 16).T
    return np.tile(w, (8, 1)).astype(np.int16)


def _plane(arr, dtype=np.float32):
    return np.ascontiguousarray(arr.reshape(-1, P).T).astype(dtype)


def _deal(blk_ids, nblk, ncores, cap_round=1):
    """Deal items to cores per block, round-robin. Returns (per-core position
    lists, caps). Items of block b on core c land at a common offset table."""
    order = np.argsort(blk_ids, kind="stable")
    counts = np.bincount(blk_ids, minlength=nblk)
    caps = (counts + ncores - 1) // ncores
    return order, counts, caps


def kernel(beta, z0, v0, a0, u, v, event_times, pair_u, pair_v, t0, tn):
    assert not np.any(np.asarray(a0)), "kernel assumes a0 == 0"
    beta = np.asarray(beta, np.float32)
    z0 = np.asarray(z0, np.float32)
    v0 = np.asarray(v0, np.float32)
    u = np.asarray(u).astype(np.int64)
    v = np.asarray(v).astype(np.int64)
    event_times = np.asarray(event_times, np.float32)
    pair_u = np.asarray(pair_u).astype(np.int64)
    pair_v = np.asarray(pair_v).astype(np.int64)
    t0f = float(np.asarray(t0))
    tnf = float(np.asarray(tn))
    b = float(beta.reshape(-1)[0])
    E = u.shape[0]
    NPAIR = pair_u.shape[0]

    # ---- deal events/pairs to cores by u-block ----
    ublk = (u // P).astype(np.int64)
    order, counts, caps = _deal(ublk, NBLK128, N_CORES)
    assert caps.sum() <= NEV_PAD, caps.sum()
    pblk = (pair_u // P).astype(np.int64)
    porder, pcounts, pcaps = _deal(pblk, NBLK128, N_CORES)
    assert pcaps.sum() <= NPR_PAD, pcaps.sum()

    ev_segs = _segments(caps, NEV_PAD)
    pr_segs = _segments(pcaps, NPR_PAD)
    key = (tuple(caps), tuple(pcaps))
    if key not in _cache:
        _cache[key] = _build(ev_segs, pr_segs)
    nc = _cache[key]

    # per-core slot assignment: block b's events at cols [off_b, off_b+cap_b)
    offs = np.zeros(NBLK128 + 1, np.int64)
    offs[1:] = np.cumsum(caps)
    poffs = np.zeros(NBLK128 + 1, np.int64)
    poffs[1:] = np.cumsum(pcaps)

    # positions: sorted events of block b occupy order[cstart_b : cstart_b+counts_b];
    # deal index j -> core j % 8, slot off_b + j // 8
    cstart = np.zeros(NBLK128 + 1, np.int64)
    cstart[1:] = np.cumsum(counts)
    pcstart = np.zeros(NBLK128 + 1, np.int64)
    pcstart[1:] = np.cumsum(pcounts)

    def placements(order_, counts_, cstart_, offs_, npad):
        """-> core id and padded position for each original item (by sorted order)."""
        n = order_.shape[0]
        j_in_blk = np.arange(n) - np.repeat(cstart_[:-1], counts_)
        cores = j_in_blk % N_CORES
        slots = np.repeat(offs_[:-1], counts_) + j_in_blk // N_CORES
        return cores, slots  # aligned with order_

    ev_cores, ev_slots = placements(order, counts, cstart, offs, NEV_PAD)
    pr_cores, pr_slots = placements(porder, pcounts, pcstart, poffs, NPR_PAD)

    # ---- tables ----
    tblb = np.zeros((P, NBLK128 * 4), np.float32)
    n = np.arange(N_POINTS)
    tblb[n % P, 4 * (n // P) + 0] = z0[:, 0]
    tblb[n % P, 4 * (n // P) + 1] = z0[:, 1]
    tblb[n % P, 4 * (n // P) + 2] = v0[:, 0]
    tblb[n % P, 4 * (n // P) + 3] = v0[:, 1]
    tblb = tblb.astype(ml_dtypes.bfloat16)

    tb32 = np.zeros((NBLK4 * 4, 16), np.float32)
    tb32[:N_POINTS, 0:2] = z0
    tb32[:N_POINTS, 2:4] = v0
    tb32 = np.ascontiguousarray(tb32.reshape(NBLK4, 64))

    dt = (tnf - t0f) / N_RIEMANN
    ts = (t0f + (np.arange(N_RIEMANN, dtype=np.float32) / N_RIEMANN)
          * (tnf - t0f)).astype(np.float32)
    t2d = np.tile(ts[None, :], (P, 1))

    in_maps = []
    for c in range(N_CORES):
        sel = ev_cores == c
        so = order[sel]              # original event ids for this core
        slots = ev_slots[sel]
        uu = np.zeros(NEV_PAD, np.int64)
        vv = np.full(NEV_PAD, 4 * (NBLK4 - 1), np.int64)  # pad -> zero block
        tt = np.zeros(NEV_PAD, np.float32)
        live = np.zeros(NEV_PAD, bool)
        uu[slots] = u[so]
        vv[slots] = v[so]
        tt[slots] = event_times[so]
        live[slots] = True
        ohu = np.zeros((P, NEV_PAD), ml_dtypes.float8_e4m3fn)
        ohu[uu[live] % P, np.flatnonzero(live)] = 1.0
        mskf = np.zeros((4, NEV_PAD), np.float32)
        mskf[vv[live] % 4, np.flatnonzero(live)] = 1.0
        msk = np.stack([_plane(mskf[s], np.int8) for s in range(4)], axis=1)

        psel = pr_cores == c
        pso = porder[psel]
        pslots = pr_slots[psel]
        pu_ = np.zeros(NPR_PAD, np.int64)
        pv_ = np.full(NPR_PAD, 4 * (NBLK4 - 1), np.int64)
        plive = np.zeros(NPR_PAD, bool)
        pu_[pslots] = pair_u[pso]
        pv_[pslots] = pair_v[pso]
        plive[pslots] = True
        ohpr = np.zeros((P, NPR_PAD), ml_dtypes.float8_e4m3fn)
        ohpr[pu_[plive] % P, np.flatnonzero(plive)] = 1.0
        pmskf = np.zeros((4, NPR_PAD), np.float32)
        pmskf[pv_[plive] % 4, np.flatnonzero(plive)] = 1.0
        pmsk = np.stack([_plane(pmskf[s], np.int8) for s in range(4)], axis=1)

        in_maps.append({
            "tblb": tblb,
            "ohu": ohu,
            "vbl": _wrap16(vv // 4),
            "te": _plane(tt),
            "msk": msk,
            "table32": tb32,
            "ohp": ohpr,
            "pvb": _wrap16(pv_ // 4),
            "pmsk": pmsk,
            "t2d": t2d,
            "t2g": np.tile(ts[None, None, :], (P, 13, 1)),
            "betac": np.full((1, 1), b, np.float32),
        })

    trace = bool(_os.environ.get("KERNEL_TRACE"))
    if trace:
        try:
            import types
            if "antenv.axon_hooks" not in _sys.modules:
                mod = types.ModuleType("antenv.axon_hooks")
                mod._hook = None
                mod.set_axon_ntff_profile_hook = lambda h: setattr(mod, "_hook", h)
                mod.get_axon_ntff_profile_hook = lambda: mod._hook
                import antenv
                antenv.axon_hooks = mod
                _sys.modules["antenv.axon_hooks"] = mod
                from trn_agent_boot.trn_boot import _ntff_profile_via_ctypes
                hk = _ntff_profile_via_ctypes("/opt/axon/libaxon_pjrt.so")
                if hk is not None:
                    mod.set_axon_ntff_profile_hook(hk)
        except Exception:
            trace = False
    from concourse.bass_utils import run_bass_kernel_spmd
    r = run_bass_kernel_spmd(nc, in_maps, core_ids=list(range(N_CORES)),
                             trace=trace)
    globals()["LAST_EXEC_NS"] = r.exec_time_ns

    ev_sum = 0.0
    ne_sum = 0.0
    for c in range(N_CORES):
        out = r.results[c]["out"].astype(np.float64)
        ev_sum += out[:, 0].sum()
        ne_sum += out[:, 1].sum()

    # pad corrections: event pad -> d = sqrt(2)*eps; pair pad -> per t step
    d_dummy = np.sqrt(2.0) * EPS
    n_ev_dummy = N_CORES * NEV_PAD - E
    n_pr_dummy = N_CORES * NPR_PAD - NPAIR
    ev_sum -= n_ev_dummy * d_dummy
    ne_sum -= n_pr_dummy * N_RIEMANN * np.exp(b - d_dummy)

    globals()["DEBUG_PARTS"] = (ev_sum, ne_sum)
    result = b * E - ev_sum - NON_EVENT_W * ne_sum * dt
    return np.float32(result)
